# revision 21
# baseline (speedup 1.0000x reference)
"""Trainium2 Bass kernel for the CTG_EPA block (dense transformer).

Shapes: x (4, 128, 16, 16, 16) -> (output (4,128,16,16,16), cgf (4,2,16,16,16)).
Sharding: 8 cores = 4 samples x 2 query-halves (sequence-parallel attention).
Each core receives its sample's full token set (rotated so its query half and
conv halo sit at fixed positions -> SPMD-uniform program) and computes:
  LayerNorm (channel-major, PE column-sum + K=1 broadcast matmuls)
  q/k/vch projections (channel-major), vc (token-major), Gram G = xn^T xn
  full NxN attention for its 2048 queries (S^T layout, softmax transpose-free)
  CxC channel attention via G, gated fusion, residual
  Conv3d C->2 k=3 SAME via 3-stage shift-fold matmuls (dx, dy, dz)
"""

import numpy as np

import concourse.bass as bass
import concourse.tile as tile
from concourse import bacc
from concourse import mybir
from concourse.bass_utils import run_bass_kernel_spmd

F32 = mybir.dt.float32
BF16 = mybir.dt.bfloat16
R32 = mybir.dt.float32r

B, C, D, H, W = 4, 128, 16, 16, 16
N = D * H * W            # 4096 tokens per sample
NQ = 2048                # queries per core
Q0 = 256                 # query offset in rotated token space
QCH = 1024               # attention query chunk
NKT = N // 128           # 32 key tiles
NCH = N // 512           # 8 LN/proj chunks
SCALE_C = 1.0 / float(np.sqrt(C))
SCALE_N = 1.0 / float(np.sqrt(N))
EPS = 1e-5

# padded conv volume (local): 10 z-slices x 18 x 18 (+2 guard)
ZP = 10
PLANE = 18 * 18          # 324
XPN = ZP * PLANE         # 3240
CONVN = 2560             # rotated tokens feeding conv (10 z-slices x 256)


def r32(ap):
    return ap.bitcast(R32)


def _build_program():
    nc = bacc.Bacc(None, target_bir_lowering=False)

    # ---- dram parameters (per-core inputs) ----
    def inp(name, shape):
        return nc.declare_dram_parameter(name, list(shape), F32, isOutput=False)[:]

    xT = inp("xT", (C, N))                 # raw x, channel-major, rotated
    WqT = inp("WqT", (C, C))               # Wq.T  (c_in, c_out)
    WkT = inp("WkT", (C, C))
    WvcT = inp("WvcT", (C, C))
    WvchT = inp("WvchT", (C, C))
    gb2 = inp("gb2", (2, C))               # rows [ln_g, ln_b]
    lnb_col = inp("lnb_col", (C, 1))
    bq_row = inp("bq_row", (1, C))
    bk_row = inp("bk_row", (1, C))
    bkN_row = inp("bkN_row", (1, C))       # bk * N
    bq_col = inp("bq_col", (C, 1))
    bk_col = inp("bk_col", (C, 1))
    bvch_col = inp("bvch_col", (C, 1))
    bvc_col = inp("bvc_col", (C, 1))
    WgT = inp("WgT", (C, 3))
    bg_row = inp("bg_row", (1, 3))
    bconv_col = inp("bconv_col", (2, 1))
    Wc = [inp(f"Wc{i}", (C, 18)) for i in range(3)]      # per-dx conv weights
    E2a = [inp(f"E2a{i}", (18, 6)) for i in range(3)]    # dy-fold selectors
    E2b = [inp(f"E2b{i}", (6, 2)) for i in range(3)]     # dz-fold selectors
    mask_lo = inp("mask_lo", (C, 256))
    mask_hi = inp("mask_hi", (C, 256))
    ident = inp("ident", (C, C))

    out_d = nc.declare_dram_parameter("out", [C, NQ], F32, isOutput=True)[:]
    cgf_d = nc.declare_dram_parameter("cgf", [2, NQ], F32, isOutput=True)[:]

    with tile.TileContext(nc) as tc:
        sb = tc.alloc_tile_pool(name="sb", bufs=1)
        sc = tc.alloc_tile_pool(name="sc", bufs=3)     # small rotating scratch
        # single PSUM pool, three tags -> 4 + 2 + 2 = 8 banks
        ps = tc.alloc_tile_pool(name="ps", bufs=1, space="PSUM")

        def psA(shape, dtype=F32, name="psA_t"):
            return ps.tile(shape, dtype, tag="A", bufs=2, name=name)

        def psB(shape, dtype=F32, name="psB_t"):
            return ps.tile(shape, dtype, tag="B", bufs=1, name=name)

        def psC(shape, dtype=F32, name="psC_t"):
            return ps.tile(shape, dtype, tag="C", bufs=1, name=name)

        # ---- persistent SBUF tensors ----
        xT_sb = sb.tile([C, N], F32)
        xn_sb = sb.tile([C, N], R32)
        kT_bf = sb.tile([C, N], BF16)
        qT_bf = sb.tile([C, NQ], BF16)
        vchT_sb = sb.tile([C, NQ], R32)
        vc_bf = sb.tile([C, N], BF16)         # token-major vc blocks
        xtok_bf = sb.tile([C, N], BF16, tag="big8k")  # token-major xn (Gram)
        xp_sb = sb.tile([C, XPN + 2], R32)    # padded conv input (+guard)
        s1_sb = sb.tile([18, XPN + 40], R32)
        s2_sb = sb.tile([6, XPN], R32)
        cgf_sb = sb.tile([2, NQ], F32)
        och_sb = sb.tile([C, NQ], F32)
        s32_sb = sb.tile([C, 32], F32)
        sq32_sb = sb.tile([C, 32], F32)
        a32_sb = sb.tile([C, 32], F32)
        d32_sb = sb.tile([C, 32], F32)
        m32_sb = sb.tile([C, 32], F32)
        G_sb = sb.tile([C, C], R32)
        T1_sb = sb.tile([C, C], R32)
        Ae_sb = sb.tile([C, C], F32)
        Asm_sb = sb.tile([C, C], R32)
        AsmT_sb = sb.tile([C, C], R32)
        wb_sb = sb.tile([C, 3], F32)

        # small constants / vectors
        ones_col = sb.tile([C, 1], F32)
        ones_bf = sb.tile([C, 1], BF16)
        ones_row = sb.tile([1, C], F32)
        eps_col = sb.tile([C, 1], F32)
        pooled = sb.tile([C, 1], F32)
        pooledm = sb.tile([C, 1], F32)
        sbar = sb.tile([C, 1], F32)
        rq_sb = sb.tile([1, C], R32)
        rk_sb = sb.tile([1, C], R32)
        asum = sb.tile([C, 1], F32)
        arec = sb.tile([C, 1], F32)
        wgr = sb.tile([1, 3], F32)
        we = sb.tile([1, 3], F32)
        ws = sb.tile([1, 1], F32)
        wrec = sb.tile([1, 1], F32)
        wn = sb.tile([1, 3], F32)
        bgr_sb = sb.tile([1, 3], F32)

        nc.vector.memset(ones_col, 1.0)
        nc.vector.memset(ones_bf, 1.0)
        nc.vector.memset(ones_row, 1.0)
        nc.vector.memset(eps_col, EPS)

        # ---- input DMAs ----
        dma = nc.sync.dma_start
        dma(out=xT_sb, in_=xT)
        WqT_sb = sb.tile([C, C], F32); dma(out=WqT_sb, in_=WqT)
        WkT_sb = sb.tile([C, C], F32); dma(out=WkT_sb, in_=WkT)
        WvcT_sb = sb.tile([C, C], F32); dma(out=WvcT_sb, in_=WvcT)
        WvchT_sb = sb.tile([C, C], F32); dma(out=WvchT_sb, in_=WvchT)
        gb2_sb = sb.tile([2, C], F32); dma(out=gb2_sb, in_=gb2)
        lnb_col_sb = sb.tile([C, 1], F32); dma(out=lnb_col_sb, in_=lnb_col)
        bqr_sb = sb.tile([1, C], F32); dma(out=bqr_sb, in_=bq_row)
        bkr_sb = sb.tile([1, C], F32); dma(out=bkr_sb, in_=bk_row)
        bkNr_sb = sb.tile([1, C], F32); dma(out=bkNr_sb, in_=bkN_row)
        bqc_sb = sb.tile([C, 1], F32); dma(out=bqc_sb, in_=bq_col)
        bkc_sb = sb.tile([C, 1], F32); dma(out=bkc_sb, in_=bk_col)
        bvchc_sb = sb.tile([C, 1], F32); dma(out=bvchc_sb, in_=bvch_col)
        bvcc_sb = sb.tile([C, 1], F32); dma(out=bvcc_sb, in_=bvc_col)
        WgT_sb = sb.tile([C, 3], F32); dma(out=WgT_sb, in_=WgT)
        dma(out=bgr_sb, in_=bg_row)
        bconv_sb = sb.tile([2, 1], F32); dma(out=bconv_sb, in_=bconv_col)
        Wc_sb = []
        for i in range(3):
            t = sb.tile([C, 18], F32, name=f"Wc_sb{i}"); dma(out=t, in_=Wc[i]); Wc_sb.append(t)
        E2a_sb = []
        for i in range(3):
            t = sb.tile([18, 6], F32, name=f"E2a_sb{i}"); dma(out=t, in_=E2a[i]); E2a_sb.append(t)
        E2b_sb = []
        for i in range(3):
            t = sb.tile([6, 2], F32, name=f"E2b_sb{i}"); dma(out=t, in_=E2b[i]); E2b_sb.append(t)
        mlo_sb = sb.tile([C, 256], F32); dma(out=mlo_sb, in_=mask_lo)
        mhi_sb = sb.tile([C, 256], F32); dma(out=mhi_sb, in_=mask_hi)
        id_sb = sb.tile([C, C], F32); dma(out=id_sb, in_=ident)
        idR_sb = sb.tile([C, C], R32)
        nc.vector.tensor_copy(out=idR_sb, in_=id_sb)

        mm = nc.tensor.matmul

        # fp32r-rounded copies of weights consumed by fp32r matmuls
        WqTr_sb = sb.tile([C, C], R32)
        nc.vector.tensor_copy(out=WqTr_sb, in_=WqT_sb)
        WkTr_sb = sb.tile([C, C], R32)
        nc.vector.tensor_copy(out=WkTr_sb, in_=WkT_sb)
        WvcTr_sb = sb.tile([C, C], R32)
        nc.vector.tensor_copy(out=WvcTr_sb, in_=WvcT_sb)
        WvchTr_sb = sb.tile([C, C], R32)
        nc.vector.tensor_copy(out=WvchTr_sb, in_=WvchT_sb)
        gb2_c = sb.tile([2, C], F32)
        nc.vector.tensor_copy(out=gb2_c, in_=gb2_sb)
        WgT_c = sb.tile([C, 3], F32)
        nc.vector.tensor_copy(out=WgT_c, in_=WgT_sb)
        WcR_sb = []
        for i in range(3):
            t = sb.tile([C, 18], R32, name=f"WcR_sb{i}")
            nc.vector.tensor_copy(out=t, in_=Wc_sb[i])
            WcR_sb.append(t)
        E2aR_sb = []
        for i in range(3):
            t = sb.tile([18, 6], R32, name=f"E2aR_sb{i}")
            nc.vector.tensor_copy(out=t, in_=E2a_sb[i])
            E2aR_sb.append(t)
        E2bR_sb = []
        for i in range(3):
            t = sb.tile([6, 2], R32, name=f"E2bR_sb{i}")
            nc.vector.tensor_copy(out=t, in_=E2b_sb[i])
            E2bR_sb.append(t)

        # ---- gating weights (independent of LN) ----
        nc.vector.tensor_reduce(out=pooled, in_=xT_sb, axis=mybir.AxisListType.X,
                                op=mybir.AluOpType.add)
        nc.vector.tensor_scalar_mul(out=pooledm, in0=pooled, scalar1=1.0 / N)
        wg_ps = psC([1, 3], name="wg_ps")
        mm(wg_ps, pooledm, WgT_c)
        nc.vector.tensor_tensor(out=wgr, in0=wg_ps, in1=bgr_sb,
                                op=mybir.AluOpType.add)
        nc.scalar.activation(out=we, in_=wgr, func=mybir.ActivationFunctionType.Exp)
        nc.vector.tensor_reduce(out=ws, in_=we, axis=mybir.AxisListType.X,
                                op=mybir.AluOpType.add)
        nc.vector.reciprocal(out=wrec, in_=ws)
        nc.vector.tensor_scalar_mul(out=wn, in0=we, scalar1=wrec)
        wb_ps = psA([C, 3], name="wb_ps")
        mm(wb_ps, ones_row, wn)
        nc.vector.tensor_copy(out=wb_sb, in_=wb_ps)

        # ---- LayerNorm over channels ----
        # column sums of x and x^2 via ones-matmul, chunked; hop to (128,32)
        # for parallel stats, hop back to rows for K=1 broadcast matmuls.
        for j in range(NCH):
            cs = slice(512 * j, 512 * (j + 1))
            xc_sc = sc.tile([C, 512], F32, tag="xc512", bufs=3, name="xc_sc")
            nc.vector.tensor_copy(out=xc_sc, in_=xT_sb[:, cs])
            x2_sc = sc.tile([C, 512], F32, tag="ch512", bufs=3, name="x2_sc")
            nc.vector.tensor_tensor(out=x2_sc, in0=xc_sc, in1=xc_sc,
                                    op=mybir.AluOpType.mult)
            sx_ps = psA([1, 512], name="sx_ps")
            mm(sx_ps, ones_col, xc_sc)
            sxr_sc = sc.tile([1, 512], F32, tag="row512", bufs=4, name="sxr_sc")
            nc.vector.tensor_copy(out=sxr_sc, in_=sx_ps)
            dma(out=s32_sb[16 * j:16 * (j + 1), :],
                in_=sxr_sc[0:1, :].rearrange("a (p f) -> a p f", p=16))
            sq_ps = psA([1, 512], name="sq_ps")
            mm(sq_ps, ones_col, x2_sc)
            sqr_sc = sc.tile([1, 512], F32, tag="row512", bufs=4, name="sqr_sc")
            nc.vector.tensor_copy(out=sqr_sc, in_=sq_ps)
            dma(out=sq32_sb[16 * j:16 * (j + 1), :],
                in_=sqr_sc[0:1, :].rearrange("a (p f) -> a p f", p=16))
        # stats: var = (Sx2 - Sx^2/C)/C ; s = 1/sqrt(var+eps) ; m = (Sx/C)*s
        nc.vector.scalar_tensor_tensor(out=a32_sb, in0=s32_sb, scalar=1.0 / C,
                                       in1=s32_sb, op0=mybir.AluOpType.mult,
                                       op1=mybir.AluOpType.mult)
        nc.vector.tensor_tensor(out=d32_sb, in0=sq32_sb, in1=a32_sb,
                                op=mybir.AluOpType.subtract)
        nc.scalar.activation(out=d32_sb, in_=d32_sb,
                             func=mybir.ActivationFunctionType.Sqrt,
                             bias=eps_col, scale=1.0 / C)
        nc.vector.reciprocal(out=a32_sb, in_=d32_sb)           # a32 <- rstd s
        nc.vector.scalar_tensor_tensor(out=m32_sb, in0=s32_sb, scalar=1.0 / C,
                                       in1=a32_sb, op0=mybir.AluOpType.mult,
                                       op1=mybir.AluOpType.mult)  # m = mu*s

        for j in range(NCH):
            cs = slice(512 * j, 512 * (j + 1))
            srow_sc = sc.tile([1, 512], F32, tag="row512", bufs=4, name="srow_sc")
            dma(out=srow_sc[0:1, :].rearrange("a (p f) -> a p f", p=16),
                in_=a32_sb[16 * j:16 * (j + 1), :])
            srow_c = sc.tile([1, 512], F32, tag="row512b", bufs=4, name="srow_c")
            nc.vector.tensor_copy(out=srow_c, in_=srow_sc)
            mrow_sc = sc.tile([1, 512], F32, tag="row512", bufs=4, name="mrow_sc")
            dma(out=mrow_sc[0:1, :].rearrange("a (p f) -> a p f", p=16),
                in_=m32_sb[16 * j:16 * (j + 1), :])
            mrow_c = sc.tile([1, 512], F32, tag="row512b", bufs=4, name="mrow_c")
            nc.vector.tensor_copy(out=mrow_c, in_=mrow_sc)
            sb_ps = psA([C, 512], name="sb_ps")
            mm(sb_ps, gb2_c[0:1, :], srow_c)                   # g_c * s_n
            mb_ps = psA([C, 512], name="mb_ps")
            mm(mb_ps, gb2_c[0:1, :], mrow_c)                   # g_c * m_n
            t_sc = sc.tile([C, 512], F32, tag="ch512", bufs=3, name="t_sc")
            nc.vector.tensor_tensor(out=t_sc, in0=xT_sb[:, cs], in1=sb_ps,
                                    op=mybir.AluOpType.mult)
            # xn = (x*g*s + b) - g*m
            nc.vector.scalar_tensor_tensor(out=xn_sb[:, cs], in0=t_sc,
                                           scalar=lnb_col_sb, in1=mb_ps,
                                           op0=mybir.AluOpType.add,
                                           op1=mybir.AluOpType.subtract)

        # ---- projections ----
        for j in range(NCH):
            cs = slice(512 * j, 512 * (j + 1))
            k_ps = psA([C, 512], name="k_ps")
            mm(k_ps, WkTr_sb, xn_sb[:, cs])
            nc.vector.tensor_scalar_add(out=kT_bf[:, cs], in0=k_ps, scalar1=bkc_sb)
        for j in range(NQ // 512):
            cs = slice(512 * j, 512 * (j + 1))
            qs = slice(Q0 + 512 * j, Q0 + 512 * (j + 1))
            q_ps = psA([C, 512], name="q_ps")
            mm(q_ps, WqTr_sb, xn_sb[:, qs])
            nc.vector.tensor_scalar_add(out=qT_bf[:, cs], in0=q_ps, scalar1=bqc_sb)
            vh_ps = psA([C, 512], name="vh_ps")
            mm(vh_ps, WvchTr_sb, xn_sb[:, qs])
            nc.vector.tensor_scalar_add(out=vchT_sb[:, cs], in0=vh_ps,
                                        scalar1=bvchc_sb)
        for j in range(NKT):
            bs = slice(128 * j, 128 * (j + 1))
            vc_ps = psA([128, C], name="vc_ps")
            mm(vc_ps, xn_sb[:, bs], WvcTr_sb)
            nc.vector.tensor_copy(out=vc_bf[:, bs], in_=vc_ps)

        # ---- Gram + channel attention matrix ----
        for j in range(NKT):
            bs = slice(128 * j, 128 * (j + 1))
            tr_ps = psA([128, C], R32, name="tr_ps")
            nc.tensor.transpose(tr_ps, xn_sb[:, bs], idR_sb)
            nc.vector.tensor_copy(out=xtok_bf[:, bs], in_=tr_ps)
        G_ps = psB([C, C], name="G_ps")
        for j in range(NKT):
            bs = slice(128 * j, 128 * (j + 1))
            mm(G_ps, xtok_bf[:, bs], xtok_bf[:, bs],
               start=(j == 0), stop=(j == NKT - 1))
        nc.vector.tensor_copy(out=G_sb, in_=G_ps)
        nc.vector.tensor_reduce(out=sbar, in_=xn_sb.bitcast(F32),
                                axis=mybir.AxisListType.X,
                                op=mybir.AluOpType.add)
        sbar_r = sb.tile([C, 1], R32)
        nc.vector.tensor_copy(out=sbar_r, in_=sbar)
        bqr_r = sb.tile([1, C], R32)
        nc.vector.tensor_copy(out=bqr_r, in_=bqr_sb)
        bkr_r = sb.tile([1, C], R32)
        nc.vector.tensor_copy(out=bkr_r, in_=bkr_sb)
        bkNr_r = sb.tile([1, C], R32)
        nc.vector.tensor_copy(out=bkNr_r, in_=bkNr_sb)
        T1_ps = psA([C, C], name="T1_ps")
        mm(T1_ps, G_sb, WkTr_sb)
        nc.vector.tensor_copy(out=T1_sb, in_=T1_ps)
        rq_ps = psA([1, C], name="rq_ps")
        mm(rq_ps, sbar_r, WqTr_sb)
        nc.vector.tensor_copy(out=rq_sb, in_=rq_ps)
        rk_ps = psA([1, C], name="rk_ps")
        mm(rk_ps, sbar_r, WkTr_sb)
        nc.vector.tensor_copy(out=rk_sb, in_=rk_ps)
        A_ps = psB([C, C], name="A_ps")
        mm(A_ps, WqTr_sb, T1_sb, start=True, stop=False)
        mm(A_ps, rq_sb, bkr_r, start=False, stop=False)
        mm(A_ps, bqr_r, rk_sb, start=False, stop=False)
        mm(A_ps, bqr_r, bkNr_r, start=False, stop=True)
        nc.scalar.activation(out=Ae_sb, in_=A_ps,
                             func=mybir.ActivationFunctionType.Exp,
                             scale=SCALE_N, accum_out=asum)
        nc.vector.reciprocal(out=arec, in_=asum)
        nc.vector.tensor_scalar_mul(out=Asm_sb, in0=Ae_sb, scalar1=arec)
        At_ps = psA([C, C], R32, name="At_ps")
        nc.tensor.transpose(At_ps, Asm_sb, idR_sb)
        nc.vector.tensor_copy(out=AsmT_sb, in_=At_ps)
        for j in range(NQ // 512):
            cs = slice(512 * j, 512 * (j + 1))
            oc_ps = psA([C, 512], name="oc_ps")
            mm(oc_ps, AsmT_sb, vchT_sb[:, cs])
            nc.vector.tensor_copy(out=och_sb[:, cs], in_=oc_ps)

        # ---- conv3d via 3-stage shift-fold ----
        zeros512 = sb.tile([C, 512], F32)
        nc.vector.memset(zeros512, 0.0)
        for ch in range((XPN + 2 + 511) // 512):
            lo = 512 * ch
            ln = min(512, XPN + 2 - lo)
            nc.vector.tensor_copy(out=xp_sb[:, lo:lo + ln], in_=zeros512[:, 0:ln])
        xp4 = xp_sb[:, 0:XPN].rearrange("c (z y x) -> c z y x", z=ZP, y=18)
        xn4 = xn_sb[:, 0:CONVN].rearrange("c (z y x) -> c z y x", z=ZP, y=16)
        for zp in range(ZP):
            dst = xp4[:, zp, 1:17, 1:17]
            if zp == 0:
                nc.vector.tensor_tensor(out=dst, in0=xn4[:, zp],
                                        in1=mlo_sb.rearrange("c (y x) -> c y x", y=16),
                                        op=mybir.AluOpType.mult)
            elif zp == ZP - 1:
                nc.vector.tensor_tensor(out=dst, in0=xn4[:, zp],
                                        in1=mhi_sb.rearrange("c (y x) -> c y x", y=16),
                                        op=mybir.AluOpType.mult)
            else:
                nc.vector.tensor_copy(out=dst, in_=xn4[:, zp])
        # stage1: fold dx (contraction over C), range XPN
        for ch in range(7):
            lo = 512 * ch
            ln = min(512, XPN - lo)
            s1_ps = psA([18, 512], name="s1_ps")
            for dx in range(3):
                mm(s1_ps[:, 0:ln], WcR_sb[dx], xp_sb[:, lo + dx:lo + dx + ln],
                   start=(dx == 0), stop=(dx == 2))
            nc.vector.tensor_copy(out=s1_sb[:, lo:lo + ln], in_=s1_ps[:, 0:ln])
        # stage2a: fold dy
        for ch in range(7):
            lo = 512 * ch
            ln = min(512, XPN - lo)
            s2_ps = psA([6, 512], name="s2_ps")
            for dy in range(3):
                mm(s2_ps[:, 0:ln], E2aR_sb[dy],
                   s1_sb[:, lo + 18 * dy:lo + 18 * dy + ln],
                   start=(dy == 0), stop=(dy == 2))
            nc.vector.tensor_copy(out=s2_sb[:, lo:lo + ln], in_=s2_ps[:, 0:ln])
        # stage2b: fold dz, per output z-slice
        s2v = s2_sb.rearrange("c (z y x) -> c z y x", z=ZP, y=18)
        for zl in range(8):
            cz_ps = psA([2, 256], name="cz_ps")
            for dz in range(3):
                mm(cz_ps.rearrange("c (y x) -> c y x", y=16),
                   E2bR_sb[dz], s2v[:, zl + dz, 0:16, 0:16],
                   start=(dz == 0), stop=(dz == 2))
            nc.vector.tensor_scalar_add(out=cgf_sb[:, 256 * zl:256 * (zl + 1)],
                                        in0=cz_ps, scalar1=bconv_sb)
        dma(out=cgf_d, in_=cgf_sb)

        # ---- attention (2 query chunks x 32 key tiles) ----
        out_sb = sb.tile([C, NQ], F32, tag="big8k")  # reuses xtok slot
        for c in range(2):
            q_lo = QCH * c
            o_ps = psB([C, QCH], name="o_ps")
            d_ps = psC([1, QCH], name="d_ps")
            for j in range(NKT):
                bs = slice(128 * j, 128 * (j + 1))
                s_ps = psA([C, QCH], name="s_ps")
                for h in range(2):
                    hs = slice(512 * h, 512 * (h + 1))
                    qs = slice(q_lo + 512 * h, q_lo + 512 * (h + 1))
                    mm(s_ps[:, hs], kT_bf[:, bs], qT_bf[:, qs])
                p_bf = sc.tile([C, QCH], BF16, tag="pbuf", bufs=3, name="p_bf")
                nc.scalar.activation(out=p_bf, in_=s_ps,
                                     func=mybir.ActivationFunctionType.Exp,
                                     scale=SCALE_C)
                for h in range(2):
                    hs = slice(512 * h, 512 * (h + 1))
                    mm(o_ps[:, hs], vc_bf[:, bs], p_bf[:, hs],
                       start=(j == 0), stop=(j == NKT - 1))
                    mm(d_ps[0:1, hs], ones_bf, p_bf[:, hs],
                       start=(j == 0), stop=(j == NKT - 1))
            # fusion: out = x + w0*((P@vc)/d + bvc) + w2*och
            dr_sc = sc.tile([1, QCH], F32, tag="dr", bufs=1, name="dr_sc")
            nc.vector.reciprocal(out=dr_sc, in_=d_ps)
            rb_ps = psA([C, QCH], name="rb_ps")
            for h in range(2):
                hs = slice(512 * h, 512 * (h + 1))
                mm(rb_ps[:, hs], ones_row, dr_sc[0:1, hs])
            rb_sc = sc.tile([C, QCH], F32, tag="rb", bufs=1, name="rb_sc")
            nc.vector.tensor_copy(out=rb_sc, in_=rb_ps)
            t1_sc = sc.tile([C, QCH], F32, tag="t1", bufs=1, name="t1_sc")
            nc.vector.tensor_tensor(out=t1_sc, in0=o_ps, in1=rb_sc,
                                    op=mybir.AluOpType.mult)
            t2_sc = sc.tile([C, QCH], F32, tag="t2", bufs=1, name="t2_sc")
            nc.vector.tensor_scalar(out=t2_sc, in0=t1_sc, scalar1=bvcc_sb,
                                    scalar2=wb_sb[:, 0:1],
                                    op0=mybir.AluOpType.add,
                                    op1=mybir.AluOpType.mult)
            t3_sc = sc.tile([C, QCH], F32, tag="t3", bufs=1, name="t3_sc")
            nc.vector.scalar_tensor_tensor(out=t3_sc, in0=och_sb[:, q_lo:q_lo + QCH],
                                           scalar=wb_sb[:, 2:3], in1=t2_sc,
                                           op0=mybir.AluOpType.mult,
                                           op1=mybir.AluOpType.add)
            nc.vector.tensor_tensor(out=out_sb[:, q_lo:q_lo + QCH], in0=t3_sc,
                                    in1=xT_sb[:, Q0 + q_lo:Q0 + q_lo + QCH],
                                    op=mybir.AluOpType.add)
            dma(out=out_d[:, q_lo:q_lo + QCH], in_=out_sb[:, q_lo:q_lo + QCH])

        ps.release()
        sc.release()
        sb.release()

    nc.finalize()
    return nc


_PROGRAM = None


def _get_program():
    global _PROGRAM
    if _PROGRAM is None:
        _PROGRAM = _build_program()
    return _PROGRAM


def _host_inputs(x, ln_g, ln_b, Wq, bq, Wk, bk, Wvc, bvc, Wvch, bvch,
                 Wconv, bconv, Wg, bg):
    """Build the 8 per-core input dicts (numpy, float32)."""
    f = np.float32
    common = {
        "WqT": np.ascontiguousarray(Wq.T, f),
        "WkT": np.ascontiguousarray(Wk.T, f),
        "WvcT": np.ascontiguousarray(Wvc.T, f),
        "WvchT": np.ascontiguousarray(Wvch.T, f),
        "gb2": np.stack([ln_g, ln_b]).astype(f),
        "lnb_col": ln_b[:, None].astype(f),
        "bq_row": bq[None, :].astype(f),
        "bk_row": bk[None, :].astype(f),
        "bkN_row": (bk * N)[None, :].astype(f),
        "bq_col": bq[:, None].astype(f),
        "bk_col": bk[:, None].astype(f),
        "bvch_col": bvch[:, None].astype(f),
        "bvc_col": bvc[:, None].astype(f),
        "WgT": np.ascontiguousarray(Wg.T, f),
        "bg_row": bg[None, :].astype(f),
        "bconv_col": bconv[:, None].astype(f),
        "ident": np.eye(C, dtype=f),
    }
    # conv tap weights: Wc_dx[c, (dz*3+dy)*2+o] = Wconv[o,c,dz,dy,dx]
    for dx in range(3):
        wt = Wconv[:, :, :, :, dx]                      # (2, C, 3, 3)
        wt = np.transpose(wt, (1, 2, 3, 0)).reshape(C, 18)
        common[f"Wc{dx}"] = np.ascontiguousarray(wt, f)
    # selectors
    for dy in range(3):
        e = np.zeros((18, 6), f)
        for dz in range(3):
            for o in range(2):
                e[(dz * 3 + dy) * 2 + o, dz * 2 + o] = 1.0
        common[f"E2a{dy}"] = e
    for dz in range(3):
        e = np.zeros((6, 2), f)
        for o in range(2):
            e[dz * 2 + o, o] = 1.0
        common[f"E2b{dz}"] = e

    maps = []
    for core in range(8):
        b, h = core // 2, core % 2
        xb = np.ascontiguousarray(x[b].reshape(C, N), f)
        shift = 256 * (8 * h - 1)
        rot = np.roll(xb, -shift, axis=1)
        m = dict(common)
        m["xT"] = np.ascontiguousarray(rot, f)
        m["mask_lo"] = np.full((C, 256), 1.0 if h == 1 else 0.0, f)
        m["mask_hi"] = np.full((C, 256), 1.0 if h == 0 else 0.0, f)
        maps.append(m)
    return maps


def kernel(**inputs):
    inputs = {k: np.asarray(v, np.float32) for k, v in inputs.items()}
    nc = _get_program()
    maps = _host_inputs(**inputs)
    res = run_bass_kernel_spmd(nc, maps, core_ids=list(range(8)))
    output = np.empty((B, C, D, H, W), np.float32)
    cgf = np.empty((B, 2, D, H, W), np.float32)
    for core in range(8):
        b, h = core // 2, core % 2
        zs = slice(8 * h, 8 * h + 8)
        output[b, :, zs] = res.results[core]["out"].reshape(C, 8, H, W)
        cgf[b, :, zs] = res.results[core]["cgf"].reshape(2, 8, H, W)
    return output, cgf


# revision 24
# speedup vs baseline: 1.2944x; 1.2944x over previous
"""Trainium2 Bass kernel for the CTG_EPA block (dense transformer).

Shapes: x (4, 128, 16, 16, 16) -> (output (4,128,16,16,16), cgf (4,2,16,16,16)).
Sharding: 8 cores = 4 samples x 2 query-halves (sequence-parallel attention).
Each core receives its sample's full token set (rotated so its query half and
conv halo sit at fixed positions -> SPMD-uniform program) and computes:
  LayerNorm (channel-major, PE column-sum + K=1 broadcast matmuls)
  q/k/vch projections (channel-major), vc (token-major), Gram G = xn^T xn
  full NxN attention for its 2048 queries (S^T layout, softmax transpose-free)
  CxC channel attention via G, gated fusion, residual
  Conv3d C->2 k=3 SAME via 3-stage shift-fold matmuls (dx, dy, dz)
"""

import numpy as np

import concourse.bass as bass
import concourse.tile as tile
from concourse import bacc
from concourse import mybir
from concourse.bass_utils import run_bass_kernel_spmd

F32 = mybir.dt.float32
BF16 = mybir.dt.bfloat16
R32 = mybir.dt.float32r

B, C, D, H, W = 4, 128, 16, 16, 16
N = D * H * W            # 4096 tokens per sample
NQ = 2048                # queries per core
Q0 = 256                 # query offset in rotated token space
QCH = 1024               # attention query chunk
NKT = N // 128           # 32 key tiles
NCH = N // 512           # 8 LN/proj chunks
SCALE_C = 1.0 / float(np.sqrt(C))
SCALE_N = 1.0 / float(np.sqrt(N))
EPS = 1e-5

# padded conv volume (local): 10 z-slices x 18 x 18 (+2 guard)
ZP = 10
PLANE = 18 * 18          # 324
XPN = ZP * PLANE         # 3240
CONVN = 2560             # rotated tokens feeding conv (10 z-slices x 256)


def r32(ap):
    return ap.bitcast(R32)


def _build_program():
    nc = bacc.Bacc(None, target_bir_lowering=False)

    # ---- dram parameters (per-core inputs) ----
    def inp(name, shape):
        return nc.declare_dram_parameter(name, list(shape), F32, isOutput=False)[:]

    xT = inp("xT", (C, N))                 # raw x, channel-major, rotated
    WqT = inp("WqT", (C, C))               # Wq.T  (c_in, c_out)
    WkT = inp("WkT", (C, C))
    WvcT = inp("WvcT", (C, C))
    WvchT = inp("WvchT", (C, C))
    gb2 = inp("gb2", (2, C))               # rows [ln_g, ln_b]
    lnb_col = inp("lnb_col", (C, 1))
    bq_row = inp("bq_row", (1, C))
    bk_row = inp("bk_row", (1, C))
    bkN_row = inp("bkN_row", (1, C))       # bk * N
    bq_col = inp("bq_col", (C, 1))
    bk_col = inp("bk_col", (C, 1))
    bvch_col = inp("bvch_col", (C, 1))
    bvc_col = inp("bvc_col", (C, 1))
    WgT = inp("WgT", (C, 3))
    bg_row = inp("bg_row", (1, 3))
    bconv_col = inp("bconv_col", (2, 1))
    Wc = [inp(f"Wc{i}", (C, 18)) for i in range(3)]      # per-dx conv weights
    E2a = [inp(f"E2a{i}", (18, 6)) for i in range(3)]    # dy-fold selectors
    E2b = [inp(f"E2b{i}", (6, 2)) for i in range(3)]     # dz-fold selectors
    mask_lo = inp("mask_lo", (C, 256))
    mask_hi = inp("mask_hi", (C, 256))
    ident = inp("ident", (C, C))

    out_d = nc.declare_dram_parameter("out", [C, NQ], F32, isOutput=True)[:]
    cgf_d = nc.declare_dram_parameter("cgf", [2, NQ], F32, isOutput=True)[:]

    with tile.TileContext(nc) as tc:
        sb = tc.alloc_tile_pool(name="sb", bufs=1)
        sc = tc.alloc_tile_pool(name="sc", bufs=3)     # small rotating scratch
        # single PSUM pool, three tags -> 4 + 2 + 2 = 8 banks
        ps = tc.alloc_tile_pool(name="ps", bufs=1, space="PSUM")

        def psA(shape, dtype=F32, name="psA_t"):
            return ps.tile(shape, dtype, tag="A", bufs=2, name=name)

        def psB(shape, dtype=F32, name="psB_t"):
            return ps.tile(shape, dtype, tag="B", bufs=1, name=name)

        def psC(shape, dtype=F32, name="psC_t"):
            return ps.tile(shape, dtype, tag="C", bufs=1, name=name)

        # ---- persistent SBUF tensors ----
        xT_sb = sb.tile([C, N], F32)
        xn_sb = sb.tile([C, N], R32)
        kT_bf = sb.tile([C, N], BF16)
        qT_bf = sb.tile([C, NQ], BF16)
        vchT_sb = sb.tile([C, NQ], R32)
        vc_bf = sb.tile([C, N], BF16)         # token-major vc blocks
        xtok_bf = sb.tile([C, N], BF16, tag="big8k")  # token-major xn (Gram)
        xp_sb = sb.tile([C, XPN + 2], R32)    # padded conv input (+guard)
        s1_sb = sb.tile([18, XPN + 40], R32)
        s2_sb = sb.tile([6, XPN], R32)
        cgf_sb = sb.tile([2, NQ], F32)
        och_sb = sb.tile([C, NQ], F32)
        xn_bf = sb.tile([C, N], BF16)
        s32_sb = sb.tile([C, 32], F32)
        sq32_sb = sb.tile([C, 32], F32)
        a32_sb = sb.tile([C, 32], F32)
        d32_sb = sb.tile([C, 32], F32)
        m32_sb = sb.tile([C, 32], F32)
        G_sb = sb.tile([C, C], R32)
        T1_sb = sb.tile([C, C], R32)
        Ae_sb = sb.tile([C, C], F32)
        Asm_sb = sb.tile([C, C], R32)
        AsmT_sb = sb.tile([C, C], R32)
        wb_sb = sb.tile([C, 3], F32)

        # small constants / vectors
        ones_col = sb.tile([C, 1], F32)
        ones_bf = sb.tile([C, 1], BF16)
        ones_row = sb.tile([1, C], F32)
        eps_col = sb.tile([C, 1], F32)
        pooled = sb.tile([C, 1], F32)
        pooledm = sb.tile([C, 1], F32)
        sbar = sb.tile([C, 1], F32)
        rq_sb = sb.tile([1, C], R32)
        rk_sb = sb.tile([1, C], R32)
        asum = sb.tile([C, 1], F32)
        arec = sb.tile([C, 1], F32)
        wgr = sb.tile([1, 3], F32)
        we = sb.tile([1, 3], F32)
        ws = sb.tile([1, 1], F32)
        wrec = sb.tile([1, 1], F32)
        wn = sb.tile([1, 3], F32)
        bgr_sb = sb.tile([1, 3], F32)

        nc.vector.memset(ones_col, 1.0)
        nc.vector.memset(ones_bf, 1.0)
        nc.vector.memset(ones_row, 1.0)
        nc.vector.memset(eps_col, EPS)

        # ---- input DMAs ----
        dma = nc.sync.dma_start
        for j in range(NCH):
            cs = slice(512 * j, 512 * (j + 1))
            dma(out=xT_sb[:, cs], in_=xT[:, cs])
        WqT_sb = sb.tile([C, C], F32); dma(out=WqT_sb, in_=WqT)
        WkT_sb = sb.tile([C, C], F32); dma(out=WkT_sb, in_=WkT)
        WvcT_sb = sb.tile([C, C], F32); dma(out=WvcT_sb, in_=WvcT)
        WvchT_sb = sb.tile([C, C], F32); dma(out=WvchT_sb, in_=WvchT)
        gb2_sb = sb.tile([2, C], F32); dma(out=gb2_sb, in_=gb2)
        lnb_col_sb = sb.tile([C, 1], F32); dma(out=lnb_col_sb, in_=lnb_col)
        bqr_sb = sb.tile([1, C], F32); dma(out=bqr_sb, in_=bq_row)
        bkr_sb = sb.tile([1, C], F32); dma(out=bkr_sb, in_=bk_row)
        bkNr_sb = sb.tile([1, C], F32); dma(out=bkNr_sb, in_=bkN_row)
        bqc_sb = sb.tile([C, 1], F32); dma(out=bqc_sb, in_=bq_col)
        bkc_sb = sb.tile([C, 1], F32); dma(out=bkc_sb, in_=bk_col)
        bvchc_sb = sb.tile([C, 1], F32); dma(out=bvchc_sb, in_=bvch_col)
        bvcc_sb = sb.tile([C, 1], F32); dma(out=bvcc_sb, in_=bvc_col)
        WgT_sb = sb.tile([C, 3], F32); dma(out=WgT_sb, in_=WgT)
        dma(out=bgr_sb, in_=bg_row)
        bconv_sb = sb.tile([2, 1], F32); dma(out=bconv_sb, in_=bconv_col)
        Wc_sb = []
        for i in range(3):
            t = sb.tile([C, 18], F32, name=f"Wc_sb{i}"); dma(out=t, in_=Wc[i]); Wc_sb.append(t)
        E2a_sb = []
        for i in range(3):
            t = sb.tile([18, 6], F32, name=f"E2a_sb{i}"); dma(out=t, in_=E2a[i]); E2a_sb.append(t)
        E2b_sb = []
        for i in range(3):
            t = sb.tile([6, 2], F32, name=f"E2b_sb{i}"); dma(out=t, in_=E2b[i]); E2b_sb.append(t)
        mlo_sb = sb.tile([C, 256], F32); dma(out=mlo_sb, in_=mask_lo)
        mhi_sb = sb.tile([C, 256], F32); dma(out=mhi_sb, in_=mask_hi)
        id_sb = sb.tile([C, C], F32); dma(out=id_sb, in_=ident)
        idR_sb = sb.tile([C, C], R32)
        nc.vector.tensor_copy(out=idR_sb, in_=id_sb)
        id_bf = sb.tile([C, C], BF16)
        nc.vector.tensor_copy(out=id_bf, in_=id_sb)

        mm = nc.tensor.matmul

        # fp32r-rounded copies of weights consumed by fp32r matmuls
        WqTr_sb = sb.tile([C, C], R32)
        nc.vector.tensor_copy(out=WqTr_sb, in_=WqT_sb)
        WkTr_sb = sb.tile([C, C], R32)
        nc.vector.tensor_copy(out=WkTr_sb, in_=WkT_sb)
        WvcT_bf = sb.tile([C, C], BF16)
        nc.vector.tensor_copy(out=WvcT_bf, in_=WvcT_sb)
        WvchTr_sb = sb.tile([C, C], R32)
        nc.vector.tensor_copy(out=WvchTr_sb, in_=WvchT_sb)
        gb2_c = sb.tile([2, C], R32)
        nc.vector.tensor_copy(out=gb2_c, in_=gb2_sb)
        ones_row_r = sb.tile([1, C], R32)
        nc.vector.tensor_copy(out=ones_row_r, in_=ones_row)
        WgT_c = sb.tile([C, 3], F32)
        nc.vector.tensor_copy(out=WgT_c, in_=WgT_sb)
        WcR_sb = []
        for i in range(3):
            t = sb.tile([C, 18], R32, name=f"WcR_sb{i}")
            nc.vector.tensor_copy(out=t, in_=Wc_sb[i])
            WcR_sb.append(t)
        E2aR_sb = []
        for i in range(3):
            t = sb.tile([18, 6], R32, name=f"E2aR_sb{i}")
            nc.vector.tensor_copy(out=t, in_=E2a_sb[i])
            E2aR_sb.append(t)
        E2bR_sb = []
        for i in range(3):
            t = sb.tile([6, 2], R32, name=f"E2bR_sb{i}")
            nc.vector.tensor_copy(out=t, in_=E2b_sb[i])
            E2bR_sb.append(t)

        # ---- gating weights (independent of LN) ----
        nc.vector.tensor_reduce(out=pooled, in_=xT_sb, axis=mybir.AxisListType.X,
                                op=mybir.AluOpType.add)
        nc.vector.tensor_scalar_mul(out=pooledm, in0=pooled, scalar1=1.0 / N)
        wg_ps = psC([1, 3], name="wg_ps")
        mm(wg_ps, pooledm, WgT_c)
        nc.vector.tensor_tensor(out=wgr, in0=wg_ps, in1=bgr_sb,
                                op=mybir.AluOpType.add)
        nc.scalar.activation(out=we, in_=wgr, func=mybir.ActivationFunctionType.Exp)
        nc.vector.tensor_reduce(out=ws, in_=we, axis=mybir.AxisListType.X,
                                op=mybir.AluOpType.add)
        nc.vector.reciprocal(out=wrec, in_=ws)
        nc.vector.tensor_scalar_mul(out=wn, in0=we, scalar1=wrec)
        wb_ps = psA([C, 3], name="wb_ps")
        mm(wb_ps, ones_row, wn)
        nc.vector.tensor_copy(out=wb_sb, in_=wb_ps)

        # ---- LayerNorm over channels ----
        # column sums of x and x^2 via ones-matmul, chunked; hop to (128,32)
        # for parallel stats, hop back to rows for K=1 broadcast matmuls.
        for j in range(NCH):
            cs = slice(512 * j, 512 * (j + 1))
            x2_sc = sc.tile([C, 512], F32, tag="ch512", bufs=3, name="x2_sc")
            nc.vector.tensor_tensor(out=x2_sc, in0=xT_sb[:, cs], in1=xT_sb[:, cs],
                                    op=mybir.AluOpType.mult)
            sx_ps = psA([1, 512], name="sx_ps")
            mm(sx_ps, ones_col, xT_sb[:, cs])
            sxr_sc = sc.tile([1, 512], F32, tag="row512", bufs=4, name="sxr_sc")
            nc.vector.tensor_copy(out=sxr_sc, in_=sx_ps)
            dma(out=s32_sb[16 * j:16 * (j + 1), :],
                in_=sxr_sc[0:1, :].rearrange("a (p f) -> a p f", p=16))
            sq_ps = psA([1, 512], name="sq_ps")
            mm(sq_ps, ones_col, x2_sc)
            sqr_sc = sc.tile([1, 512], F32, tag="row512", bufs=4, name="sqr_sc")
            nc.vector.tensor_copy(out=sqr_sc, in_=sq_ps)
            dma(out=sq32_sb[16 * j:16 * (j + 1), :],
                in_=sqr_sc[0:1, :].rearrange("a (p f) -> a p f", p=16))
        # stats: var = (Sx2 - Sx^2/C)/C ; s = 1/sqrt(var+eps) ; m = (Sx/C)*s
        nc.vector.scalar_tensor_tensor(out=a32_sb, in0=s32_sb, scalar=1.0 / C,
                                       in1=s32_sb, op0=mybir.AluOpType.mult,
                                       op1=mybir.AluOpType.mult)
        nc.vector.tensor_tensor(out=d32_sb, in0=sq32_sb, in1=a32_sb,
                                op=mybir.AluOpType.subtract)
        nc.scalar.activation(out=d32_sb, in_=d32_sb,
                             func=mybir.ActivationFunctionType.Sqrt,
                             bias=eps_col, scale=1.0 / C)
        nc.vector.reciprocal(out=a32_sb, in_=d32_sb)           # a32 <- rstd s
        nc.vector.scalar_tensor_tensor(out=m32_sb, in0=s32_sb, scalar=1.0 / C,
                                       in1=a32_sb, op0=mybir.AluOpType.mult,
                                       op1=mybir.AluOpType.mult)  # m = mu*s

        for j in range(NCH):
            cs = slice(512 * j, 512 * (j + 1))
            srow_sc = sc.tile([1, 512], F32, tag="row512", bufs=4, name="srow_sc")
            dma(out=srow_sc[0:1, :].rearrange("a (p f) -> a p f", p=16),
                in_=a32_sb[16 * j:16 * (j + 1), :])
            srow_c = sc.tile([1, 512], R32, tag="row512b", bufs=4, name="srow_c")
            nc.vector.tensor_copy(out=srow_c, in_=srow_sc)
            mrow_sc = sc.tile([1, 512], F32, tag="row512", bufs=4, name="mrow_sc")
            dma(out=mrow_sc[0:1, :].rearrange("a (p f) -> a p f", p=16),
                in_=m32_sb[16 * j:16 * (j + 1), :])
            mrow_c = sc.tile([1, 512], R32, tag="row512b", bufs=4, name="mrow_c")
            nc.vector.tensor_copy(out=mrow_c, in_=mrow_sc)
            sb_ps = psA([C, 512], name="sb_ps")
            mm(sb_ps, gb2_c[0:1, :], srow_c)                   # g_c * s_n
            mb_ps = psA([C, 512], name="mb_ps")
            mm(mb_ps, gb2_c[0:1, :], mrow_c)                   # g_c * m_n
            t_sc = sc.tile([C, 512], F32, tag="ch512", bufs=3, name="t_sc")
            nc.vector.tensor_tensor(out=t_sc, in0=xT_sb[:, cs], in1=sb_ps,
                                    op=mybir.AluOpType.mult)
            # xn = (x*g*s + b) - g*m
            nc.vector.scalar_tensor_tensor(out=xn_sb[:, cs], in0=t_sc,
                                           scalar=lnb_col_sb, in1=mb_ps,
                                           op0=mybir.AluOpType.add,
                                           op1=mybir.AluOpType.subtract)
            nc.vector.tensor_copy(out=xn_bf[:, cs], in_=xn_sb[:, cs])

        # ---- projections ----
        for j in range(NCH):
            cs = slice(512 * j, 512 * (j + 1))
            k_ps = psA([C, 512], name="k_ps")
            mm(k_ps, WkTr_sb, xn_sb[:, cs])
            nc.vector.tensor_scalar_add(out=kT_bf[:, cs], in0=k_ps, scalar1=bkc_sb)
        for j in range(NQ // 512):
            cs = slice(512 * j, 512 * (j + 1))
            qs = slice(Q0 + 512 * j, Q0 + 512 * (j + 1))
            q_ps = psA([C, 512], name="q_ps")
            mm(q_ps, WqTr_sb, xn_sb[:, qs])
            nc.vector.tensor_scalar_add(out=qT_bf[:, cs], in0=q_ps, scalar1=bqc_sb)
            vh_ps = psA([C, 512], name="vh_ps")
            mm(vh_ps, WvchTr_sb, xn_sb[:, qs])
            nc.vector.tensor_scalar_add(out=vchT_sb[:, cs], in0=vh_ps,
                                        scalar1=bvchc_sb)
        for j in range(NKT):
            bs = slice(128 * j, 128 * (j + 1))
            vc_ps = psA([128, C], name="vc_ps")
            mm(vc_ps, xn_bf[:, bs], WvcT_bf)
            nc.vector.tensor_copy(out=vc_bf[:, bs], in_=vc_ps)

        # ---- Gram + channel attention matrix ----
        for j in range(NKT):
            bs = slice(128 * j, 128 * (j + 1))
            tr_ps = psA([128, C], BF16, name="tr_ps")
            nc.tensor.transpose(tr_ps, xn_bf[:, bs], id_bf)
            nc.vector.tensor_copy(out=xtok_bf[:, bs], in_=tr_ps)
        G_ps = psB([C, C], name="G_ps")
        for j in range(NKT):
            bs = slice(128 * j, 128 * (j + 1))
            mm(G_ps, xtok_bf[:, bs], xtok_bf[:, bs],
               start=(j == 0), stop=(j == NKT - 1))
        nc.vector.tensor_copy(out=G_sb, in_=G_ps)
        nc.vector.tensor_reduce(out=sbar, in_=xn_sb.bitcast(F32),
                                axis=mybir.AxisListType.X,
                                op=mybir.AluOpType.add)
        sbar_r = sb.tile([C, 1], R32)
        nc.vector.tensor_copy(out=sbar_r, in_=sbar)
        bqr_r = sb.tile([1, C], R32)
        nc.vector.tensor_copy(out=bqr_r, in_=bqr_sb)
        bkr_r = sb.tile([1, C], R32)
        nc.vector.tensor_copy(out=bkr_r, in_=bkr_sb)
        bkNr_r = sb.tile([1, C], R32)
        nc.vector.tensor_copy(out=bkNr_r, in_=bkNr_sb)
        T1_ps = psA([C, C], name="T1_ps")
        mm(T1_ps, G_sb, WkTr_sb)
        nc.vector.tensor_copy(out=T1_sb, in_=T1_ps)
        rq_ps = psA([1, C], name="rq_ps")
        mm(rq_ps, sbar_r, WqTr_sb)
        nc.vector.tensor_copy(out=rq_sb, in_=rq_ps)
        rk_ps = psA([1, C], name="rk_ps")
        mm(rk_ps, sbar_r, WkTr_sb)
        nc.vector.tensor_copy(out=rk_sb, in_=rk_ps)
        A_ps = psB([C, C], name="A_ps")
        mm(A_ps, WqTr_sb, T1_sb, start=True, stop=False)
        mm(A_ps, rq_sb, bkr_r, start=False, stop=False)
        mm(A_ps, bqr_r, rk_sb, start=False, stop=False)
        mm(A_ps, bqr_r, bkNr_r, start=False, stop=True)
        nc.scalar.activation(out=Ae_sb, in_=A_ps,
                             func=mybir.ActivationFunctionType.Exp,
                             scale=SCALE_N, accum_out=asum)
        nc.vector.reciprocal(out=arec, in_=asum)
        nc.vector.tensor_scalar_mul(out=Asm_sb, in0=Ae_sb, scalar1=arec)
        At_ps = psA([C, C], R32, name="At_ps")
        nc.tensor.transpose(At_ps, Asm_sb, idR_sb)
        nc.vector.tensor_copy(out=AsmT_sb, in_=At_ps)
        for j in range(NQ // 512):
            cs = slice(512 * j, 512 * (j + 1))
            oc_ps = psA([C, 512], name="oc_ps")
            mm(oc_ps, AsmT_sb, vchT_sb[:, cs])
            nc.vector.tensor_copy(out=och_sb[:, cs], in_=oc_ps)

        # ---- conv3d via 3-stage shift-fold ----
        zeros512 = sb.tile([C, 512], F32)
        nc.vector.memset(zeros512, 0.0)
        for ch in range((XPN + 2 + 511) // 512):
            lo = 512 * ch
            ln = min(512, XPN + 2 - lo)
            nc.vector.tensor_copy(out=xp_sb[:, lo:lo + ln], in_=zeros512[:, 0:ln])
        xp4 = xp_sb[:, 0:XPN].rearrange("c (z y x) -> c z y x", z=ZP, y=18)
        xn4 = xn_sb[:, 0:CONVN].rearrange("c (z y x) -> c z y x", z=ZP, y=16)
        for zp in range(ZP):
            dst = xp4[:, zp, 1:17, 1:17]
            if zp == 0:
                nc.vector.tensor_tensor(out=dst, in0=xn4[:, zp],
                                        in1=mlo_sb.rearrange("c (y x) -> c y x", y=16),
                                        op=mybir.AluOpType.mult)
            elif zp == ZP - 1:
                nc.vector.tensor_tensor(out=dst, in0=xn4[:, zp],
                                        in1=mhi_sb.rearrange("c (y x) -> c y x", y=16),
                                        op=mybir.AluOpType.mult)
            else:
                nc.vector.tensor_copy(out=dst, in_=xn4[:, zp])
        # stage1: fold dx (contraction over C), range XPN
        for ch in range(7):
            lo = 512 * ch
            ln = min(512, XPN - lo)
            s1_ps = psA([18, 512], name="s1_ps")
            for dx in range(3):
                mm(s1_ps[:, 0:ln], WcR_sb[dx], xp_sb[:, lo + dx:lo + dx + ln],
                   start=(dx == 0), stop=(dx == 2))
            nc.vector.tensor_copy(out=s1_sb[:, lo:lo + ln], in_=s1_ps[:, 0:ln])
        # stage2a: fold dy
        for ch in range(7):
            lo = 512 * ch
            ln = min(512, XPN - lo)
            s2_ps = psA([6, 512], name="s2_ps")
            for dy in range(3):
                mm(s2_ps[:, 0:ln], E2aR_sb[dy],
                   s1_sb[:, lo + 18 * dy:lo + 18 * dy + ln],
                   start=(dy == 0), stop=(dy == 2))
            nc.vector.tensor_copy(out=s2_sb[:, lo:lo + ln], in_=s2_ps[:, 0:ln])
        # stage2b: fold dz, per output z-slice
        s2v = s2_sb.rearrange("c (z y x) -> c z y x", z=ZP, y=18)
        for zl in range(8):
            cz_ps = psA([2, 256], name="cz_ps")
            for dz in range(3):
                mm(cz_ps.rearrange("c (y x) -> c y x", y=16),
                   E2bR_sb[dz], s2v[:, zl + dz, 0:16, 0:16],
                   start=(dz == 0), stop=(dz == 2))
            nc.vector.tensor_scalar_add(out=cgf_sb[:, 256 * zl:256 * (zl + 1)],
                                        in0=cz_ps, scalar1=bconv_sb)
        dma(out=cgf_d, in_=cgf_sb)

        # ---- attention (2 query chunks x 32 key tiles) ----
        out_sb = sb.tile([C, NQ], F32, tag="big8k")  # reuses xtok slot
        for c in range(2):
            q_lo = QCH * c
            o_ps = psB([C, QCH], name="o_ps")
            d_ps = psC([1, QCH], name="d_ps")
            p_prev = None
            for j in range(NKT):
                bs = slice(128 * j, 128 * (j + 1))
                s_ps = psA([C, QCH], name="s_ps")
                for h in range(2):
                    hs = slice(512 * h, 512 * (h + 1))
                    qs = slice(q_lo + 512 * h, q_lo + 512 * (h + 1))
                    mm(s_ps[:, hs], kT_bf[:, bs], qT_bf[:, qs])
                p_bf = sc.tile([C, QCH], BF16, tag="pbuf", bufs=4, name="p_bf")
                nc.scalar.activation(out=p_bf, in_=s_ps,
                                     func=mybir.ActivationFunctionType.Exp,
                                     scale=SCALE_C)
                for h in range(2):
                    hs = slice(512 * h, 512 * (h + 1))
                    mm(o_ps[:, hs], vc_bf[:, bs], p_bf[:, hs],
                       start=(j == 0), stop=(j == NKT - 1))
                if j % 2 == 0:
                    p_prev = p_bf
                else:
                    padd = sc.tile([C, QCH], BF16, tag="padd", bufs=2, name="padd")
                    with nc.allow_low_precision("pair-sum of exp tiles; relative "
                                                "error ~1e-4 on softmax denom"):
                        nc.vector.tensor_tensor(out=padd, in0=p_prev, in1=p_bf,
                                                op=mybir.AluOpType.add)
                    for h in range(2):
                        hs = slice(512 * h, 512 * (h + 1))
                        mm(d_ps[0:1, hs], ones_bf, padd[:, hs],
                           start=(j == 1), stop=(j == NKT - 1))
            # fusion: out = x + w0*((P@vc)/d + bvc) + w2*och
            dr_sc = sc.tile([1, QCH], R32, tag="dr", bufs=1, name="dr_sc")
            with nc.allow_low_precision("1/denom rounded to fp32r for the "
                                        "broadcast matmul; ~1e-4 relative"):
                nc.vector.reciprocal(out=dr_sc, in_=d_ps)
            rb_ps = psA([C, QCH], name="rb_ps")
            for h in range(2):
                hs = slice(512 * h, 512 * (h + 1))
                mm(rb_ps[:, hs], ones_row_r, dr_sc[0:1, hs])
            rb_sc = sc.tile([C, QCH], F32, tag="rb", bufs=1, name="rb_sc")
            nc.vector.tensor_copy(out=rb_sc, in_=rb_ps)
            t1_sc = sc.tile([C, QCH], F32, tag="t1", bufs=1, name="t1_sc")
            nc.vector.tensor_tensor(out=t1_sc, in0=o_ps, in1=rb_sc,
                                    op=mybir.AluOpType.mult)
            t2_sc = sc.tile([C, QCH], F32, tag="t2", bufs=1, name="t2_sc")
            nc.vector.tensor_scalar(out=t2_sc, in0=t1_sc, scalar1=bvcc_sb,
                                    scalar2=wb_sb[:, 0:1],
                                    op0=mybir.AluOpType.add,
                                    op1=mybir.AluOpType.mult)
            t3_sc = sc.tile([C, QCH], F32, tag="t3", bufs=1, name="t3_sc")
            nc.vector.scalar_tensor_tensor(out=t3_sc, in0=och_sb[:, q_lo:q_lo + QCH],
                                           scalar=wb_sb[:, 2:3], in1=t2_sc,
                                           op0=mybir.AluOpType.mult,
                                           op1=mybir.AluOpType.add)
            nc.vector.tensor_tensor(out=out_sb[:, q_lo:q_lo + QCH], in0=t3_sc,
                                    in1=xT_sb[:, Q0 + q_lo:Q0 + q_lo + QCH],
                                    op=mybir.AluOpType.add)
            dma(out=out_d[:, q_lo:q_lo + QCH], in_=out_sb[:, q_lo:q_lo + QCH])

        ps.release()
        sc.release()
        sb.release()

    nc.finalize()
    return nc


_PROGRAM = None


def _get_program():
    global _PROGRAM
    if _PROGRAM is None:
        _PROGRAM = _build_program()
    return _PROGRAM


def _host_inputs(x, ln_g, ln_b, Wq, bq, Wk, bk, Wvc, bvc, Wvch, bvch,
                 Wconv, bconv, Wg, bg):
    """Build the 8 per-core input dicts (numpy, float32)."""
    f = np.float32
    common = {
        "WqT": np.ascontiguousarray(Wq.T, f),
        "WkT": np.ascontiguousarray(Wk.T, f),
        "WvcT": np.ascontiguousarray(Wvc.T, f),
        "WvchT": np.ascontiguousarray(Wvch.T, f),
        "gb2": np.stack([ln_g, ln_b]).astype(f),
        "lnb_col": ln_b[:, None].astype(f),
        "bq_row": bq[None, :].astype(f),
        "bk_row": bk[None, :].astype(f),
        "bkN_row": (bk * N)[None, :].astype(f),
        "bq_col": bq[:, None].astype(f),
        "bk_col": bk[:, None].astype(f),
        "bvch_col": bvch[:, None].astype(f),
        "bvc_col": bvc[:, None].astype(f),
        "WgT": np.ascontiguousarray(Wg.T, f),
        "bg_row": bg[None, :].astype(f),
        "bconv_col": bconv[:, None].astype(f),
        "ident": np.eye(C, dtype=f),
    }
    # conv tap weights: Wc_dx[c, (dz*3+dy)*2+o] = Wconv[o,c,dz,dy,dx]
    for dx in range(3):
        wt = Wconv[:, :, :, :, dx]                      # (2, C, 3, 3)
        wt = np.transpose(wt, (1, 2, 3, 0)).reshape(C, 18)
        common[f"Wc{dx}"] = np.ascontiguousarray(wt, f)
    # selectors
    for dy in range(3):
        e = np.zeros((18, 6), f)
        for dz in range(3):
            for o in range(2):
                e[(dz * 3 + dy) * 2 + o, dz * 2 + o] = 1.0
        common[f"E2a{dy}"] = e
    for dz in range(3):
        e = np.zeros((6, 2), f)
        for o in range(2):
            e[dz * 2 + o, o] = 1.0
        common[f"E2b{dz}"] = e

    maps = []
    for core in range(8):
        b, h = core // 2, core % 2
        xb = np.ascontiguousarray(x[b].reshape(C, N), f)
        shift = 256 * (8 * h - 1)
        rot = np.roll(xb, -shift, axis=1)
        m = dict(common)
        m["xT"] = np.ascontiguousarray(rot, f)
        m["mask_lo"] = np.full((C, 256), 1.0 if h == 1 else 0.0, f)
        m["mask_hi"] = np.full((C, 256), 1.0 if h == 0 else 0.0, f)
        maps.append(m)
    return maps


def kernel(**inputs):
    inputs = {k: np.asarray(v, np.float32) for k, v in inputs.items()}
    nc = _get_program()
    maps = _host_inputs(**inputs)
    res = run_bass_kernel_spmd(nc, maps, core_ids=list(range(8)))
    output = np.empty((B, C, D, H, W), np.float32)
    cgf = np.empty((B, 2, D, H, W), np.float32)
    for core in range(8):
        b, h = core // 2, core % 2
        zs = slice(8 * h, 8 * h + 8)
        output[b, :, zs] = res.results[core]["out"].reshape(C, 8, H, W)
        cgf[b, :, zs] = res.results[core]["cgf"].reshape(2, 8, H, W)
    return output, cgf


# revision 27
# speedup vs baseline: 1.3026x; 1.0063x over previous
"""Trainium2 Bass kernel for the CTG_EPA block (dense transformer).

Shapes: x (4, 128, 16, 16, 16) -> (output (4,128,16,16,16), cgf (4,2,16,16,16)).
Sharding: 8 cores = 4 samples x 2 query-halves (sequence-parallel attention).
Each core receives its sample's full token set (rotated so its query half and
conv halo sit at fixed positions -> SPMD-uniform program) and computes:
  LayerNorm (channel-major, PE column-sum + K=1 broadcast matmuls)
  q/k/vch projections (channel-major), vc (token-major), Gram G = xn^T xn
  full NxN attention for its 2048 queries (S^T layout, softmax transpose-free)
  CxC channel attention via G, gated fusion, residual
  Conv3d C->2 k=3 SAME via 3-stage shift-fold matmuls (dx, dy, dz)
"""

import numpy as np

import concourse.bass as bass
import concourse.tile as tile
from concourse import bacc
from concourse import mybir
from concourse.bass_utils import run_bass_kernel_spmd

F32 = mybir.dt.float32
BF16 = mybir.dt.bfloat16
R32 = mybir.dt.float32r

B, C, D, H, W = 4, 128, 16, 16, 16
N = D * H * W            # 4096 tokens per sample
NQ = 2048                # queries per core
Q0 = 256                 # query offset in rotated token space
QCH = 1024               # attention query chunk
NKT = N // 128           # 32 key tiles
NCH = N // 512           # 8 LN/proj chunks
SCALE_C = 1.0 / float(np.sqrt(C))
SCALE_N = 1.0 / float(np.sqrt(N))
EPS = 1e-5

# padded conv volume (local): 10 z-slices x 18 x 18 (+2 guard)
ZP = 10
PLANE = 18 * 18          # 324
XPN = ZP * PLANE         # 3240
CONVN = 2560             # rotated tokens feeding conv (10 z-slices x 256)


def r32(ap):
    return ap.bitcast(R32)


def _build_program():
    nc = bacc.Bacc(None, target_bir_lowering=False)

    # ---- dram parameters (per-core inputs) ----
    def inp(name, shape):
        return nc.declare_dram_parameter(name, list(shape), F32, isOutput=False)[:]

    xT = inp("xT", (C, N))                 # raw x, channel-major, rotated
    wpack = inp("wpack", (C, 1214))        # packed 128-row weights/constants
    spack = inp("spack", (18, 540))        # packed small-row weights/constants

    out_d = nc.declare_dram_parameter("out", [C, NQ], F32, isOutput=True)[:]
    cgf_d = nc.declare_dram_parameter("cgf", [2, NQ], F32, isOutput=True)[:]

    with tile.TileContext(nc) as tc:
        sb = tc.alloc_tile_pool(name="sb", bufs=1)
        sc = tc.alloc_tile_pool(name="sc", bufs=3)     # small rotating scratch
        # single PSUM pool, three tags -> 4 + 2 + 2 = 8 banks
        ps = tc.alloc_tile_pool(name="ps", bufs=1, space="PSUM")

        def psA(shape, dtype=F32, name="psA_t"):
            return ps.tile(shape, dtype, tag="A", bufs=2, name=name)

        def psB(shape, dtype=F32, name="psB_t"):
            return ps.tile(shape, dtype, tag="B", bufs=1, name=name)

        def psC(shape, dtype=F32, name="psC_t"):
            return ps.tile(shape, dtype, tag="C", bufs=1, name=name)

        # ---- persistent SBUF tensors ----
        xT_sb = sb.tile([C, N], F32)
        xn_sb = sb.tile([C, N], R32)
        kT_bf = sb.tile([C, N], BF16)
        qT_bf = sb.tile([C, NQ], BF16)
        vc_bf = sb.tile([C, N], BF16)         # token-major vc blocks
        xtok_bf = sb.tile([C, N], BF16, tag="big8k")  # token-major xn (Gram)
        xp_sb = sb.tile([C, XPN + 2], R32)    # padded conv input (+guard)
        s1_sb = sb.tile([18, XPN + 40], R32)
        s2_sb = sb.tile([6, XPN], R32)
        cgf_sb = sb.tile([2, NQ], F32)
        och_sb = sb.tile([C, NQ], F32)
        xn_bf = sb.tile([C, N], BF16)
        sxrow_sb = sb.tile([1, N], F32)
        sqrow_sb = sb.tile([1, N], F32)
        s32_sb = sb.tile([C, 32], F32)
        sq32_sb = sb.tile([C, 32], F32)
        a32_sb = sb.tile([C, 32], F32)
        d32_sb = sb.tile([C, 32], F32)
        m32_sb = sb.tile([C, 32], F32)
        G_sb = sb.tile([C, C], R32)
        T1_sb = sb.tile([C, C], R32)
        Ae_sb = sb.tile([C, C], F32)
        Asm_sb = sb.tile([C, C], R32)
        AsmT_sb = sb.tile([C, C], R32)
        wb_sb = sb.tile([C, 3], F32)

        # small constants / vectors
        ones_col = sb.tile([C, 1], F32)
        ones_bf = sb.tile([C, 1], BF16)
        ones_row = sb.tile([1, C], F32)
        eps_col = sb.tile([C, 1], F32)
        pooled = sb.tile([C, 1], F32)
        pooledm = sb.tile([C, 1], F32)
        sbar = sb.tile([C, 1], F32)
        rq_sb = sb.tile([1, C], R32)
        rk_sb = sb.tile([1, C], R32)
        asum = sb.tile([C, 1], F32)
        arec = sb.tile([C, 1], F32)
        wgr = sb.tile([1, 3], F32)
        we = sb.tile([1, 3], F32)
        ws = sb.tile([1, 1], F32)
        wrec = sb.tile([1, 1], F32)
        wn = sb.tile([1, 3], F32)

        nc.vector.memset(ones_col, 1.0)
        nc.vector.memset(ones_bf, 1.0)
        nc.vector.memset(ones_row, 1.0)
        nc.vector.memset(eps_col, EPS)

        # ---- input DMAs ----
        dma = nc.sync.dma_start
        for j in range(NCH):
            cs = slice(512 * j, 512 * (j + 1))
            dma(out=xT_sb[:, cs], in_=xT[:, cs])
        wp_sb = sb.tile([C, 1214], F32); dma(out=wp_sb, in_=wpack)
        sp_sb = sb.tile([18, 540], F32); dma(out=sp_sb, in_=spack)
        WqT_sb = wp_sb[:, 0:128]
        WkT_sb = wp_sb[:, 128:256]
        WvcT_sb = wp_sb[:, 256:384]
        WvchT_sb = wp_sb[:, 384:512]
        id_sb = wp_sb[:, 512:640]
        WgT_sb = wp_sb[:, 640:643]
        Wc_sb = [wp_sb[:, 643 + 18 * i:661 + 18 * i] for i in range(3)]
        mlo_sb = wp_sb[:, 697:953]
        mhi_sb = wp_sb[:, 953:1209]
        lnb_col_sb = wp_sb[:, 1209:1210]
        bqc_sb = wp_sb[:, 1210:1211]
        bkc_sb = wp_sb[:, 1211:1212]
        bvchc_sb = wp_sb[:, 1212:1213]
        bvcc_sb = wp_sb[:, 1213:1214]
        E2a_sb = [sp_sb[0:18, 6 * i:6 * i + 6] for i in range(3)]
        E2b_sb = [sp_sb[0:6, 18 + 2 * i:20 + 2 * i] for i in range(3)]
        gb2_sb = sp_sb[0:2, 24:152]
        bqr_sb = sp_sb[0:1, 152:280]
        bkr_sb = sp_sb[0:1, 280:408]
        bkNr_sb = sp_sb[0:1, 408:536]
        bgr_sb = sp_sb[0:1, 536:539]
        bconv_sb = sp_sb[0:2, 539:540]
        idR_sb = sb.tile([C, C], R32)
        nc.vector.tensor_copy(out=idR_sb, in_=id_sb)
        id_bf = sb.tile([C, C], BF16)
        nc.vector.tensor_copy(out=id_bf, in_=id_sb)

        mm = nc.tensor.matmul

        # fp32r-rounded copies of weights consumed by fp32r matmuls
        WqTr_sb = sb.tile([C, C], R32)
        nc.vector.tensor_copy(out=WqTr_sb, in_=WqT_sb)
        WkTr_sb = sb.tile([C, C], R32)
        nc.vector.tensor_copy(out=WkTr_sb, in_=WkT_sb)
        WvcT_bf = sb.tile([C, C], BF16)
        nc.vector.tensor_copy(out=WvcT_bf, in_=WvcT_sb)
        WvchTr_sb = sb.tile([C, C], R32)
        nc.vector.tensor_copy(out=WvchTr_sb, in_=WvchT_sb)
        ones_row_r = sb.tile([1, C], R32)
        nc.vector.tensor_copy(out=ones_row_r, in_=ones_row)
        WcR_sb = []
        for i in range(3):
            t = sb.tile([C, 18], R32, name=f"WcR_sb{i}")
            nc.vector.tensor_copy(out=t, in_=Wc_sb[i])
            WcR_sb.append(t)
        E2aR_sb = []
        for i in range(3):
            t = sb.tile([18, 6], R32, name=f"E2aR_sb{i}")
            nc.vector.tensor_copy(out=t, in_=E2a_sb[i])
            E2aR_sb.append(t)
        E2bR_sb = []
        for i in range(3):
            t = sb.tile([6, 2], R32, name=f"E2bR_sb{i}")
            nc.vector.tensor_copy(out=t, in_=E2b_sb[i])
            E2bR_sb.append(t)

        # ---- gating weights (independent of LN) ----
        nc.vector.tensor_reduce(out=pooled, in_=xT_sb, axis=mybir.AxisListType.X,
                                op=mybir.AluOpType.add)
        nc.vector.tensor_scalar_mul(out=pooledm, in0=pooled, scalar1=1.0 / N)
        wg_ps = psC([1, 3], name="wg_ps")
        mm(wg_ps, pooledm, WgT_sb)
        nc.vector.tensor_tensor(out=wgr, in0=wg_ps, in1=bgr_sb,
                                op=mybir.AluOpType.add)
        nc.scalar.activation(out=we, in_=wgr, func=mybir.ActivationFunctionType.Exp)
        nc.vector.tensor_reduce(out=ws, in_=we, axis=mybir.AxisListType.X,
                                op=mybir.AluOpType.add)
        nc.vector.reciprocal(out=wrec, in_=ws)
        nc.vector.tensor_scalar_mul(out=wn, in0=we, scalar1=wrec)
        wb_ps = psA([C, 3], name="wb_ps")
        mm(wb_ps, ones_row, wn)
        nc.vector.tensor_copy(out=wb_sb, in_=wb_ps)

        # ---- LayerNorm over channels ----
        # column sums of x and x^2 via ones-matmul, chunked; hop to (128,32)
        # for parallel stats, hop back to rows for K=1 broadcast matmuls.
        for j in range(NCH):
            cs = slice(512 * j, 512 * (j + 1))
            x2_sc = sc.tile([C, 512], F32, tag="ch512", bufs=3, name="x2_sc")
            nc.vector.tensor_tensor(out=x2_sc, in0=xT_sb[:, cs], in1=xT_sb[:, cs],
                                    op=mybir.AluOpType.mult)
            sx_ps = psA([1, 512], name="sx_ps")
            mm(sx_ps, ones_col, xT_sb[:, cs])
            nc.vector.tensor_copy(out=sxrow_sb[:, cs], in_=sx_ps)
            sq_ps = psA([1, 512], name="sq_ps")
            mm(sq_ps, ones_col, x2_sc)
            nc.vector.tensor_copy(out=sqrow_sb[:, cs], in_=sq_ps)
        dma(out=s32_sb, in_=sxrow_sb[0:1, :].rearrange("a (p f) -> a p f", p=128))
        dma(out=sq32_sb, in_=sqrow_sb[0:1, :].rearrange("a (p f) -> a p f", p=128))
        # stats: var = (Sx2 - Sx^2/C)/C ; s = 1/sqrt(var+eps) ; m = (Sx/C)*s
        nc.vector.scalar_tensor_tensor(out=a32_sb, in0=s32_sb, scalar=1.0 / C,
                                       in1=s32_sb, op0=mybir.AluOpType.mult,
                                       op1=mybir.AluOpType.mult)
        nc.vector.tensor_tensor(out=d32_sb, in0=sq32_sb, in1=a32_sb,
                                op=mybir.AluOpType.subtract)
        nc.scalar.activation(out=d32_sb, in_=d32_sb,
                             func=mybir.ActivationFunctionType.Sqrt,
                             bias=eps_col, scale=1.0 / C)
        nc.vector.reciprocal(out=a32_sb, in_=d32_sb)           # a32 <- rstd s
        nc.vector.scalar_tensor_tensor(out=m32_sb, in0=s32_sb, scalar=1.0 / C,
                                       in1=a32_sb, op0=mybir.AluOpType.mult,
                                       op1=mybir.AluOpType.mult)  # m = mu*s
        dma(out=sxrow_sb[0:1, :].rearrange("a (p f) -> a p f", p=128), in_=a32_sb)
        dma(out=sqrow_sb[0:1, :].rearrange("a (p f) -> a p f", p=128), in_=m32_sb)

        for j in range(NCH):
            cs = slice(512 * j, 512 * (j + 1))
            sb_ps = psA([C, 512], name="sb_ps")
            mm(sb_ps, gb2_sb[0:1, :], sxrow_sb[:, cs])         # g_c * s_n
            mb_ps = psA([C, 512], name="mb_ps")
            mm(mb_ps, gb2_sb[0:1, :], sqrow_sb[:, cs])         # g_c * m_n
            t_sc = sc.tile([C, 512], F32, tag="ch512", bufs=3, name="t_sc")
            nc.vector.tensor_tensor(out=t_sc, in0=xT_sb[:, cs], in1=sb_ps,
                                    op=mybir.AluOpType.mult)
            # xn = (x*g*s + b) - g*m
            nc.vector.scalar_tensor_tensor(out=xn_sb[:, cs], in0=t_sc,
                                           scalar=lnb_col_sb, in1=mb_ps,
                                           op0=mybir.AluOpType.add,
                                           op1=mybir.AluOpType.subtract)
            nc.vector.tensor_copy(out=xn_bf[:, cs], in_=xn_sb[:, cs])

        # ---- projections ----
        for j in range(NCH):
            cs = slice(512 * j, 512 * (j + 1))
            k_ps = psA([C, 512], name="k_ps")
            mm(k_ps, WkTr_sb, xn_sb[:, cs])
            nc.vector.tensor_scalar_add(out=kT_bf[:, cs], in0=k_ps, scalar1=bkc_sb)
        for j in range(NQ // 512):
            cs = slice(512 * j, 512 * (j + 1))
            qs = slice(Q0 + 512 * j, Q0 + 512 * (j + 1))
            q_ps = psA([C, 512], name="q_ps")
            mm(q_ps, WqTr_sb, xn_sb[:, qs])
            nc.vector.tensor_scalar_add(out=qT_bf[:, cs], in0=q_ps, scalar1=bqc_sb)
        for j in range(NKT):
            bs = slice(128 * j, 128 * (j + 1))
            vc_ps = psA([128, C], name="vc_ps")
            mm(vc_ps, xn_bf[:, bs], WvcT_bf)
            nc.vector.tensor_copy(out=vc_bf[:, bs], in_=vc_ps)

        # ---- Gram + channel attention matrix ----
        for j in range(NKT):
            bs = slice(128 * j, 128 * (j + 1))
            tr_ps = psA([128, C], BF16, name="tr_ps")
            nc.tensor.transpose(tr_ps, xn_bf[:, bs], id_bf)
            nc.vector.tensor_copy(out=xtok_bf[:, bs], in_=tr_ps)
        G_ps = psB([C, C], name="G_ps")
        for j in range(NKT):
            bs = slice(128 * j, 128 * (j + 1))
            mm(G_ps, xtok_bf[:, bs], xtok_bf[:, bs],
               start=(j == 0), stop=(j == NKT - 1))
        nc.vector.tensor_copy(out=G_sb, in_=G_ps)
        nc.vector.tensor_reduce(out=sbar, in_=xn_sb.bitcast(F32),
                                axis=mybir.AxisListType.X,
                                op=mybir.AluOpType.add)
        sbar_r = sb.tile([C, 1], R32)
        nc.vector.tensor_copy(out=sbar_r, in_=sbar)
        bqr_r = sb.tile([1, C], R32)
        nc.vector.tensor_copy(out=bqr_r, in_=bqr_sb)
        bkr_r = sb.tile([1, C], R32)
        nc.vector.tensor_copy(out=bkr_r, in_=bkr_sb)
        bkNr_r = sb.tile([1, C], R32)
        nc.vector.tensor_copy(out=bkNr_r, in_=bkNr_sb)
        T1_ps = psA([C, C], name="T1_ps")
        mm(T1_ps, G_sb, WkTr_sb)
        nc.vector.tensor_copy(out=T1_sb, in_=T1_ps)
        rq_ps = psA([1, C], name="rq_ps")
        mm(rq_ps, sbar_r, WqTr_sb)
        nc.vector.tensor_copy(out=rq_sb, in_=rq_ps)
        rk_ps = psA([1, C], name="rk_ps")
        mm(rk_ps, sbar_r, WkTr_sb)
        nc.vector.tensor_copy(out=rk_sb, in_=rk_ps)
        A_ps = psB([C, C], name="A_ps")
        mm(A_ps, WqTr_sb, T1_sb, start=True, stop=False)
        mm(A_ps, rq_sb, bkr_r, start=False, stop=False)
        mm(A_ps, bqr_r, rk_sb, start=False, stop=False)
        mm(A_ps, bqr_r, bkNr_r, start=False, stop=True)
        nc.scalar.activation(out=Ae_sb, in_=A_ps,
                             func=mybir.ActivationFunctionType.Exp,
                             scale=SCALE_N, accum_out=asum)
        nc.vector.reciprocal(out=arec, in_=asum)
        nc.vector.tensor_scalar_mul(out=Asm_sb, in0=Ae_sb, scalar1=arec)
        At_ps = psA([C, C], R32, name="At_ps")
        nc.tensor.transpose(At_ps, Asm_sb, idR_sb)
        nc.vector.tensor_copy(out=AsmT_sb, in_=At_ps)
        for j in range(NQ // 512):
            cs = slice(512 * j, 512 * (j + 1))
            qs = slice(Q0 + 512 * j, Q0 + 512 * (j + 1))
            vh_ps = psA([C, 512], name="vh_ps")
            mm(vh_ps, WvchTr_sb, xn_sb[:, qs])
            vch_sc = sc.tile([C, 512], R32, tag="vch512", bufs=2, name="vch_sc")
            nc.vector.tensor_scalar_add(out=vch_sc, in0=vh_ps, scalar1=bvchc_sb)
            oc_ps = psA([C, 512], name="oc_ps")
            mm(oc_ps, AsmT_sb, vch_sc)
            nc.vector.tensor_copy(out=och_sb[:, cs], in_=oc_ps)

        # ---- conv3d via 3-stage shift-fold ----
        zeros512 = sb.tile([C, 512], F32)
        nc.vector.memset(zeros512, 0.0)
        for ch in range((XPN + 2 + 511) // 512):
            lo = 512 * ch
            ln = min(512, XPN + 2 - lo)
            nc.vector.tensor_copy(out=xp_sb[:, lo:lo + ln], in_=zeros512[:, 0:ln])
        xp4 = xp_sb[:, 0:XPN].rearrange("c (z y x) -> c z y x", z=ZP, y=18)
        xn4 = xn_sb[:, 0:CONVN].rearrange("c (z y x) -> c z y x", z=ZP, y=16)
        for zp in range(ZP):
            dst = xp4[:, zp, 1:17, 1:17]
            if zp == 0:
                nc.vector.tensor_tensor(out=dst, in0=xn4[:, zp],
                                        in1=mlo_sb.rearrange("c (y x) -> c y x", y=16),
                                        op=mybir.AluOpType.mult)
            elif zp == ZP - 1:
                nc.vector.tensor_tensor(out=dst, in0=xn4[:, zp],
                                        in1=mhi_sb.rearrange("c (y x) -> c y x", y=16),
                                        op=mybir.AluOpType.mult)
            else:
                nc.vector.tensor_copy(out=dst, in_=xn4[:, zp])
        # stage1: fold dx (contraction over C), range XPN
        for ch in range(7):
            lo = 512 * ch
            ln = min(512, XPN - lo)
            s1_ps = psA([18, 512], name="s1_ps")
            for dx in range(3):
                mm(s1_ps[:, 0:ln], WcR_sb[dx], xp_sb[:, lo + dx:lo + dx + ln],
                   start=(dx == 0), stop=(dx == 2))
            nc.vector.tensor_copy(out=s1_sb[:, lo:lo + ln], in_=s1_ps[:, 0:ln])
        # stage2a: fold dy
        for ch in range(7):
            lo = 512 * ch
            ln = min(512, XPN - lo)
            s2_ps = psA([6, 512], name="s2_ps")
            for dy in range(3):
                mm(s2_ps[:, 0:ln], E2aR_sb[dy],
                   s1_sb[:, lo + 18 * dy:lo + 18 * dy + ln],
                   start=(dy == 0), stop=(dy == 2))
            nc.vector.tensor_copy(out=s2_sb[:, lo:lo + ln], in_=s2_ps[:, 0:ln])
        # stage2b: fold dz, per output z-slice
        s2v = s2_sb.rearrange("c (z y x) -> c z y x", z=ZP, y=18)
        for zl in range(0, 8, 2):
            cz_ps = psA([2, 512], name="cz_ps")
            for dz in range(3):
                mm(cz_ps.rearrange("c (z y x) -> c z y x", z=2, y=16),
                   E2bR_sb[dz], s2v[:, zl + dz:zl + dz + 2, 0:16, 0:16],
                   start=(dz == 0), stop=(dz == 2))
            nc.vector.tensor_scalar_add(out=cgf_sb[:, 256 * zl:256 * (zl + 2)],
                                        in0=cz_ps, scalar1=bconv_sb)
        dma(out=cgf_d, in_=cgf_sb)

        # ---- attention (2 query chunks x 32 key tiles) ----
        out_sb = sb.tile([C, NQ], F32, tag="big8k")  # reuses xtok slot
        for c in range(2):
            q_lo = QCH * c
            o_ps = psB([C, QCH], name="o_ps")
            d_ps = psC([1, QCH], name="d_ps")
            p_prev = None
            for j in range(NKT):
                bs = slice(128 * j, 128 * (j + 1))
                s_ps = psA([C, QCH], name="s_ps")
                for h in range(2):
                    hs = slice(512 * h, 512 * (h + 1))
                    qs = slice(q_lo + 512 * h, q_lo + 512 * (h + 1))
                    mm(s_ps[:, hs], kT_bf[:, bs], qT_bf[:, qs])
                p_bf = sc.tile([C, QCH], BF16, tag="pbuf", bufs=3, name="p_bf")
                nc.scalar.activation(out=p_bf, in_=s_ps,
                                     func=mybir.ActivationFunctionType.Exp,
                                     scale=SCALE_C)
                for h in range(2):
                    hs = slice(512 * h, 512 * (h + 1))
                    mm(o_ps[:, hs], vc_bf[:, bs], p_bf[:, hs],
                       start=(j == 0), stop=(j == NKT - 1))
                if j % 2 == 0:
                    p_prev = p_bf
                else:
                    padd = sc.tile([C, QCH], BF16, tag="padd", bufs=2, name="padd")
                    with nc.allow_low_precision("pair-sum of exp tiles; relative "
                                                "error ~1e-4 on softmax denom"):
                        nc.vector.tensor_tensor(out=padd, in0=p_prev, in1=p_bf,
                                                op=mybir.AluOpType.add)
                    for h in range(2):
                        hs = slice(512 * h, 512 * (h + 1))
                        mm(d_ps[0:1, hs], ones_bf, padd[:, hs],
                           start=(j == 1), stop=(j == NKT - 1))
            # fusion: out = x + w0*((P@vc)/d + bvc) + w2*och
            dr_sc = sc.tile([1, QCH], R32, tag="dr", bufs=1, name="dr_sc")
            with nc.allow_low_precision("1/denom rounded to fp32r for the "
                                        "broadcast matmul; ~1e-4 relative"):
                nc.vector.reciprocal(out=dr_sc, in_=d_ps)
            rb_ps = psA([C, QCH], name="rb_ps")
            for h in range(2):
                hs = slice(512 * h, 512 * (h + 1))
                mm(rb_ps[:, hs], ones_row_r, dr_sc[0:1, hs])
            rb_sc = sc.tile([C, QCH], F32, tag="fuseA", bufs=1, name="rb_sc")
            nc.vector.tensor_copy(out=rb_sc, in_=rb_ps)
            t1_sc = sc.tile([C, QCH], F32, tag="fuseB", bufs=1, name="t1_sc")
            nc.vector.tensor_tensor(out=t1_sc, in0=o_ps, in1=rb_sc,
                                    op=mybir.AluOpType.mult)
            t2_sc = sc.tile([C, QCH], F32, tag="fuseA", bufs=1, name="t2_sc")
            nc.vector.tensor_scalar(out=t2_sc, in0=t1_sc, scalar1=bvcc_sb,
                                    scalar2=wb_sb[:, 0:1],
                                    op0=mybir.AluOpType.add,
                                    op1=mybir.AluOpType.mult)
            t3_sc = sc.tile([C, QCH], F32, tag="fuseB", bufs=1, name="t3_sc")
            nc.vector.scalar_tensor_tensor(out=t3_sc, in0=och_sb[:, q_lo:q_lo + QCH],
                                           scalar=wb_sb[:, 2:3], in1=t2_sc,
                                           op0=mybir.AluOpType.mult,
                                           op1=mybir.AluOpType.add)
            nc.vector.tensor_tensor(out=out_sb[:, q_lo:q_lo + QCH], in0=t3_sc,
                                    in1=xT_sb[:, Q0 + q_lo:Q0 + q_lo + QCH],
                                    op=mybir.AluOpType.add)
            dma(out=out_d[:, q_lo:q_lo + QCH], in_=out_sb[:, q_lo:q_lo + QCH])

        ps.release()
        sc.release()
        sb.release()

    nc.finalize()
    return nc


_PROGRAM = None


def _get_program():
    global _PROGRAM
    if _PROGRAM is None:
        _PROGRAM = _build_program()
    return _PROGRAM


def _host_inputs(x, ln_g, ln_b, Wq, bq, Wk, bk, Wvc, bvc, Wvch, bvch,
                 Wconv, bconv, Wg, bg):
    """Build the 8 per-core input dicts (numpy, float32)."""
    f = np.float32
    wpack = np.zeros((C, 1214), f)
    wpack[:, 0:128] = Wq.T
    wpack[:, 128:256] = Wk.T
    wpack[:, 256:384] = Wvc.T
    wpack[:, 384:512] = Wvch.T
    wpack[:, 512:640] = np.eye(C, dtype=f)
    wpack[:, 640:643] = Wg.T
    for dx in range(3):
        wt = np.transpose(Wconv[:, :, :, :, dx], (1, 2, 3, 0)).reshape(C, 18)
        wpack[:, 643 + 18 * dx:661 + 18 * dx] = wt
    # masks filled per-core below (cols 697:1209)
    wpack[:, 1209] = ln_b
    wpack[:, 1210] = bq
    wpack[:, 1211] = bk
    wpack[:, 1212] = bvch
    wpack[:, 1213] = bvc

    spack = np.zeros((18, 540), f)
    for dy in range(3):
        for dz in range(3):
            for o in range(2):
                spack[(dz * 3 + dy) * 2 + o, 6 * dy + dz * 2 + o] = 1.0
    for dz in range(3):
        for o in range(2):
            spack[dz * 2 + o, 18 + 2 * dz + o] = 1.0
    spack[0, 24:152] = ln_g
    spack[1, 24:152] = ln_b
    spack[0, 152:280] = bq
    spack[0, 280:408] = bk
    spack[0, 408:536] = bk * N
    spack[0, 536:539] = bg
    spack[0:2, 539] = bconv

    maps = []
    for core in range(8):
        b, h = core // 2, core % 2
        xb = np.ascontiguousarray(x[b].reshape(C, N), f)
        shift = 256 * (8 * h - 1)
        rot = np.roll(xb, -shift, axis=1)
        wp = wpack.copy()
        wp[:, 697:953] = 1.0 if h == 1 else 0.0    # mask_lo
        wp[:, 953:1209] = 1.0 if h == 0 else 0.0   # mask_hi
        maps.append({"xT": np.ascontiguousarray(rot, f), "wpack": wp,
                     "spack": spack})
    return maps


def kernel(**inputs):
    inputs = {k: np.asarray(v, np.float32) for k, v in inputs.items()}
    nc = _get_program()
    maps = _host_inputs(**inputs)
    res = run_bass_kernel_spmd(nc, maps, core_ids=list(range(8)))
    output = np.empty((B, C, D, H, W), np.float32)
    cgf = np.empty((B, 2, D, H, W), np.float32)
    for core in range(8):
        b, h = core // 2, core % 2
        zs = slice(8 * h, 8 * h + 8)
        output[b, :, zs] = res.results[core]["out"].reshape(C, 8, H, W)
        cgf[b, :, zs] = res.results[core]["cgf"].reshape(2, 8, H, W)
    return output, cgf


# revision 28
# speedup vs baseline: 1.3591x; 1.0434x over previous
"""Trainium2 Bass kernel for the CTG_EPA block (dense transformer).

Shapes: x (4, 128, 16, 16, 16) -> (output (4,128,16,16,16), cgf (4,2,16,16,16)).
Sharding: 8 cores = 4 samples x 2 query-halves (sequence-parallel attention).
Each core receives its sample's full token set (rotated so its query half and
conv halo sit at fixed positions -> SPMD-uniform program) and computes:
  LayerNorm (channel-major, PE column-sum + K=1 broadcast matmuls)
  q/k/vch projections (channel-major), vc (token-major), Gram G = xn^T xn
  full NxN attention for its 2048 queries (S^T layout, softmax transpose-free)
  CxC channel attention via G, gated fusion, residual
  Conv3d C->2 k=3 SAME via 3-stage shift-fold matmuls (dx, dy, dz)
"""

import numpy as np

import concourse.bass as bass
import concourse.tile as tile
from concourse import bacc
from concourse import mybir
from concourse.bass_utils import run_bass_kernel_spmd

F32 = mybir.dt.float32
BF16 = mybir.dt.bfloat16
R32 = mybir.dt.float32r

B, C, D, H, W = 4, 128, 16, 16, 16
N = D * H * W            # 4096 tokens per sample
NQ = 2048                # queries per core
Q0 = 256                 # query offset in rotated token space
QCH = 1024               # attention query chunk
NKT = N // 128           # 32 key tiles
NCH = N // 512           # 8 LN/proj chunks
SCALE_C = 1.0 / float(np.sqrt(C))
SCALE_N = 1.0 / float(np.sqrt(N))
EPS = 1e-5

# padded conv volume (local): 10 z-slices x 18 x 18 (+2 guard)
ZP = 10
PLANE = 18 * 18          # 324
XPN = ZP * PLANE         # 3240
CONVN = 2560             # rotated tokens feeding conv (10 z-slices x 256)


def r32(ap):
    return ap.bitcast(R32)


def _build_program():
    nc = bacc.Bacc(None, target_bir_lowering=False)

    # ---- dram parameters (per-core inputs) ----
    def inp(name, shape):
        return nc.declare_dram_parameter(name, list(shape), F32, isOutput=False)[:]

    xT = inp("xT", (C, N))                 # raw x, channel-major, rotated
    wpack = inp("wpack", (C, 1214))        # packed 128-row weights/constants
    spack = inp("spack", (18, 540))        # packed small-row weights/constants

    out_d = nc.declare_dram_parameter("out", [C, NQ], F32, isOutput=True)[:]
    cgf_d = nc.declare_dram_parameter("cgf", [2, NQ], F32, isOutput=True)[:]

    with tile.TileContext(nc) as tc:
        sb = tc.alloc_tile_pool(name="sb", bufs=1)
        sc = tc.alloc_tile_pool(name="sc", bufs=3)     # small rotating scratch
        # single PSUM pool, three tags -> 4 + 2 + 2 = 8 banks
        ps = tc.alloc_tile_pool(name="ps", bufs=1, space="PSUM")

        def psA(shape, dtype=F32, name="psA_t"):
            return ps.tile(shape, dtype, tag="A", bufs=2, name=name)

        def psB(shape, dtype=F32, name="psB_t"):
            return ps.tile(shape, dtype, tag="B", bufs=1, name=name)

        def psC(shape, dtype=F32, name="psC_t"):
            return ps.tile(shape, dtype, tag="C", bufs=1, name=name)

        # ---- persistent SBUF tensors ----
        xT_sb = sb.tile([C, N], F32)
        xn_sb = sb.tile([C, N], R32)
        kT_bf = sb.tile([C, N], BF16)
        qT_bf = sb.tile([C, NQ], BF16)
        vc_bf = sb.tile([C, N], BF16)         # token-major vc blocks
        xtok_bf = sb.tile([C, N], BF16, tag="big8k")  # token-major xn (Gram)
        xp_sb = sb.tile([C, XPN + 2], R32)    # padded conv input (+guard)
        s1_sb = sb.tile([18, XPN + 40], R32)
        s2_sb = sb.tile([6, XPN], R32)
        cgf_sb = sb.tile([2, NQ], F32)
        och_sb = sb.tile([C, NQ], F32)
        xn_bf = sb.tile([C, N], BF16)
        sxrow_sb = sb.tile([1, N], F32)
        sqrow_sb = sb.tile([1, N], F32)
        s32_sb = sb.tile([C, 32], F32)
        sq32_sb = sb.tile([C, 32], F32)
        a32_sb = sb.tile([C, 32], F32)
        d32_sb = sb.tile([C, 32], F32)
        m32_sb = sb.tile([C, 32], F32)
        G_sb = sb.tile([C, C], R32)
        T1_sb = sb.tile([C, C], R32)
        Ae_sb = sb.tile([C, C], F32)
        Asm_sb = sb.tile([C, C], R32)
        AsmT_sb = sb.tile([C, C], R32)
        wb_sb = sb.tile([C, 3], F32)

        # small constants / vectors
        ones_col = sb.tile([C, 1], F32)
        ones_bf = sb.tile([C, 1], BF16)
        ones_row = sb.tile([1, C], F32)
        eps_col = sb.tile([C, 1], F32)
        pooled = sb.tile([C, 1], F32)
        pooledm = sb.tile([C, 1], F32)
        sbar = sb.tile([C, 1], F32)
        rq_sb = sb.tile([1, C], R32)
        rk_sb = sb.tile([1, C], R32)
        asum = sb.tile([C, 1], F32)
        arec = sb.tile([C, 1], F32)
        wgr = sb.tile([1, 3], F32)
        we = sb.tile([1, 3], F32)
        ws = sb.tile([1, 1], F32)
        wrec = sb.tile([1, 1], F32)
        wn = sb.tile([1, 3], F32)

        nc.vector.memset(ones_col, 1.0)
        nc.vector.memset(ones_bf, 1.0)
        nc.vector.memset(ones_row, 1.0)
        nc.vector.memset(eps_col, EPS)

        # ---- input DMAs ----
        dma = nc.sync.dma_start
        for j in range(NCH):
            cs = slice(512 * j, 512 * (j + 1))
            dma(out=xT_sb[:, cs], in_=xT[:, cs])
        wp_sb = sb.tile([C, 1214], F32); dma(out=wp_sb, in_=wpack)
        sp_sb = sb.tile([18, 540], F32); dma(out=sp_sb, in_=spack)
        WqT_sb = wp_sb[:, 0:128]
        WkT_sb = wp_sb[:, 128:256]
        WvcT_sb = wp_sb[:, 256:384]
        WvchT_sb = wp_sb[:, 384:512]
        id_sb = wp_sb[:, 512:640]
        WgT_sb = wp_sb[:, 640:643]
        Wc_sb = [wp_sb[:, 643 + 18 * i:661 + 18 * i] for i in range(3)]
        mlo_sb = wp_sb[:, 697:953]
        mhi_sb = wp_sb[:, 953:1209]
        lnb_col_sb = wp_sb[:, 1209:1210]
        bqc_sb = wp_sb[:, 1210:1211]
        bkc_sb = wp_sb[:, 1211:1212]
        bvchc_sb = wp_sb[:, 1212:1213]
        bvcc_sb = wp_sb[:, 1213:1214]
        E2a_sb = [sp_sb[0:18, 6 * i:6 * i + 6] for i in range(3)]
        E2b_sb = [sp_sb[0:6, 18 + 2 * i:20 + 2 * i] for i in range(3)]
        gb2_sb = sp_sb[0:2, 24:152]
        bqr_sb = sp_sb[0:1, 152:280]
        bkr_sb = sp_sb[0:1, 280:408]
        bkNr_sb = sp_sb[0:1, 408:536]
        bgr_sb = sp_sb[0:1, 536:539]
        bconv_sb = sp_sb[0:2, 539:540]
        idR_sb = sb.tile([C, C], R32)
        nc.vector.tensor_copy(out=idR_sb, in_=id_sb)
        id_bf = sb.tile([C, C], BF16)
        nc.vector.tensor_copy(out=id_bf, in_=id_sb)

        mm = nc.tensor.matmul

        # fp32r-rounded copies of weights consumed by fp32r matmuls
        WqTr_sb = sb.tile([C, C], R32)
        nc.vector.tensor_copy(out=WqTr_sb, in_=WqT_sb)
        WkTr_sb = sb.tile([C, C], R32)
        nc.vector.tensor_copy(out=WkTr_sb, in_=WkT_sb)
        WvcT_bf = sb.tile([C, C], BF16)
        nc.vector.tensor_copy(out=WvcT_bf, in_=WvcT_sb)
        WvchTr_sb = sb.tile([C, C], R32)
        nc.vector.tensor_copy(out=WvchTr_sb, in_=WvchT_sb)
        ones_row_r = sb.tile([1, C], R32)
        nc.vector.tensor_copy(out=ones_row_r, in_=ones_row)
        WcR_sb = []
        for i in range(3):
            t = sb.tile([C, 18], R32, name=f"WcR_sb{i}")
            nc.vector.tensor_copy(out=t, in_=Wc_sb[i])
            WcR_sb.append(t)
        E2aR_sb = []
        for i in range(3):
            t = sb.tile([18, 6], R32, name=f"E2aR_sb{i}")
            nc.vector.tensor_copy(out=t, in_=E2a_sb[i])
            E2aR_sb.append(t)
        E2bR_sb = []
        for i in range(3):
            t = sb.tile([6, 2], R32, name=f"E2bR_sb{i}")
            nc.vector.tensor_copy(out=t, in_=E2b_sb[i])
            E2bR_sb.append(t)

        # ---- LayerNorm over channels ----
        # column sums of x and x^2 via ones-matmul, chunked; hop to (128,32)
        # for parallel stats, hop back to rows for K=1 broadcast matmuls.
        for j in range(NCH):
            cs = slice(512 * j, 512 * (j + 1))
            x2_sc = sc.tile([C, 512], F32, tag="ch512", bufs=3, name="x2_sc")
            nc.vector.tensor_tensor(out=x2_sc, in0=xT_sb[:, cs], in1=xT_sb[:, cs],
                                    op=mybir.AluOpType.mult)
            sx_ps = psA([1, 512], name="sx_ps")
            mm(sx_ps, ones_col, xT_sb[:, cs])
            nc.vector.tensor_copy(out=sxrow_sb[:, cs], in_=sx_ps)
            sq_ps = psA([1, 512], name="sq_ps")
            mm(sq_ps, ones_col, x2_sc)
            nc.vector.tensor_copy(out=sqrow_sb[:, cs], in_=sq_ps)
        dma(out=s32_sb, in_=sxrow_sb[0:1, :].rearrange("a (p f) -> a p f", p=128))
        dma(out=sq32_sb, in_=sqrow_sb[0:1, :].rearrange("a (p f) -> a p f", p=128))
        # stats: var = (Sx2 - Sx^2/C)/C ; s = 1/sqrt(var+eps) ; m = (Sx/C)*s
        nc.vector.scalar_tensor_tensor(out=a32_sb, in0=s32_sb, scalar=1.0 / C,
                                       in1=s32_sb, op0=mybir.AluOpType.mult,
                                       op1=mybir.AluOpType.mult)
        nc.vector.tensor_tensor(out=d32_sb, in0=sq32_sb, in1=a32_sb,
                                op=mybir.AluOpType.subtract)
        nc.scalar.activation(out=d32_sb, in_=d32_sb,
                             func=mybir.ActivationFunctionType.Sqrt,
                             bias=eps_col, scale=1.0 / C)
        nc.vector.reciprocal(out=a32_sb, in_=d32_sb)           # a32 <- rstd s
        nc.vector.scalar_tensor_tensor(out=m32_sb, in0=s32_sb, scalar=1.0 / C,
                                       in1=a32_sb, op0=mybir.AluOpType.mult,
                                       op1=mybir.AluOpType.mult)  # m = mu*s
        dma(out=sxrow_sb[0:1, :].rearrange("a (p f) -> a p f", p=128), in_=a32_sb)
        dma(out=sqrow_sb[0:1, :].rearrange("a (p f) -> a p f", p=128), in_=m32_sb)

        for j in range(NCH):
            cs = slice(512 * j, 512 * (j + 1))
            sb_ps = psA([C, 512], name="sb_ps")
            mm(sb_ps, gb2_sb[0:1, :], sxrow_sb[:, cs])         # g_c * s_n
            mb_ps = psA([C, 512], name="mb_ps")
            mm(mb_ps, gb2_sb[0:1, :], sqrow_sb[:, cs])         # g_c * m_n
            t_sc = sc.tile([C, 512], F32, tag="ch512", bufs=3, name="t_sc")
            nc.vector.tensor_tensor(out=t_sc, in0=xT_sb[:, cs], in1=sb_ps,
                                    op=mybir.AluOpType.mult)
            # xn = (x*g*s + b) - g*m
            nc.vector.scalar_tensor_tensor(out=xn_sb[:, cs], in0=t_sc,
                                           scalar=lnb_col_sb, in1=mb_ps,
                                           op0=mybir.AluOpType.add,
                                           op1=mybir.AluOpType.subtract)
            nc.vector.tensor_copy(out=xn_bf[:, cs], in_=xn_sb[:, cs])

        # ---- projections ----
        for j in range(NCH):
            cs = slice(512 * j, 512 * (j + 1))
            k_ps = psA([C, 512], name="k_ps")
            mm(k_ps, WkTr_sb, xn_sb[:, cs])
            nc.vector.tensor_scalar_add(out=kT_bf[:, cs], in0=k_ps, scalar1=bkc_sb)
        for j in range(NQ // 512):
            cs = slice(512 * j, 512 * (j + 1))
            qs = slice(Q0 + 512 * j, Q0 + 512 * (j + 1))
            q_ps = psA([C, 512], name="q_ps")
            mm(q_ps, WqTr_sb, xn_sb[:, qs])
            nc.vector.tensor_scalar_add(out=qT_bf[:, cs], in0=q_ps, scalar1=bqc_sb)
        for j in range(NKT):
            bs = slice(128 * j, 128 * (j + 1))
            vc_ps = psA([128, C], name="vc_ps")
            mm(vc_ps, xn_bf[:, bs], WvcT_bf)
            nc.vector.tensor_copy(out=vc_bf[:, bs], in_=vc_ps)

        # ---- Gram + channel attention matrix ----
        for j in range(NKT):
            bs = slice(128 * j, 128 * (j + 1))
            tr_ps = psA([128, C], BF16, name="tr_ps")
            nc.tensor.transpose(tr_ps, xn_bf[:, bs], id_bf)
            nc.vector.tensor_copy(out=xtok_bf[:, bs], in_=tr_ps)
        G_ps = psB([C, C], name="G_ps")
        for j in range(NKT):
            bs = slice(128 * j, 128 * (j + 1))
            mm(G_ps, xtok_bf[:, bs], xtok_bf[:, bs],
               start=(j == 0), stop=(j == NKT - 1))
        nc.vector.tensor_copy(out=G_sb, in_=G_ps)
        nc.vector.tensor_reduce(out=sbar, in_=xn_sb.bitcast(F32),
                                axis=mybir.AxisListType.X,
                                op=mybir.AluOpType.add)
        sbar_r = sb.tile([C, 1], R32)
        nc.vector.tensor_copy(out=sbar_r, in_=sbar)
        bqr_r = sb.tile([1, C], R32)
        nc.vector.tensor_copy(out=bqr_r, in_=bqr_sb)
        bkr_r = sb.tile([1, C], R32)
        nc.vector.tensor_copy(out=bkr_r, in_=bkr_sb)
        bkNr_r = sb.tile([1, C], R32)
        nc.vector.tensor_copy(out=bkNr_r, in_=bkNr_sb)
        T1_ps = psA([C, C], name="T1_ps")
        mm(T1_ps, G_sb, WkTr_sb)
        nc.vector.tensor_copy(out=T1_sb, in_=T1_ps)
        rq_ps = psA([1, C], name="rq_ps")
        mm(rq_ps, sbar_r, WqTr_sb)
        nc.vector.tensor_copy(out=rq_sb, in_=rq_ps)
        rk_ps = psA([1, C], name="rk_ps")
        mm(rk_ps, sbar_r, WkTr_sb)
        nc.vector.tensor_copy(out=rk_sb, in_=rk_ps)
        A_ps = psB([C, C], name="A_ps")
        mm(A_ps, WqTr_sb, T1_sb, start=True, stop=False)
        mm(A_ps, rq_sb, bkr_r, start=False, stop=False)
        mm(A_ps, bqr_r, rk_sb, start=False, stop=False)
        mm(A_ps, bqr_r, bkNr_r, start=False, stop=True)
        nc.scalar.activation(out=Ae_sb, in_=A_ps,
                             func=mybir.ActivationFunctionType.Exp,
                             scale=SCALE_N, accum_out=asum)
        nc.vector.reciprocal(out=arec, in_=asum)
        nc.vector.tensor_scalar_mul(out=Asm_sb, in0=Ae_sb, scalar1=arec)
        At_ps = psA([C, C], R32, name="At_ps")
        nc.tensor.transpose(At_ps, Asm_sb, idR_sb)
        nc.vector.tensor_copy(out=AsmT_sb, in_=At_ps)
        for j in range(NQ // 512):
            cs = slice(512 * j, 512 * (j + 1))
            qs = slice(Q0 + 512 * j, Q0 + 512 * (j + 1))
            vh_ps = psA([C, 512], name="vh_ps")
            mm(vh_ps, WvchTr_sb, xn_sb[:, qs])
            vch_sc = sc.tile([C, 512], R32, tag="vch512", bufs=2, name="vch_sc")
            nc.vector.tensor_scalar_add(out=vch_sc, in0=vh_ps, scalar1=bvchc_sb)
            oc_ps = psA([C, 512], name="oc_ps")
            mm(oc_ps, AsmT_sb, vch_sc)
            nc.vector.tensor_copy(out=och_sb[:, cs], in_=oc_ps)

        # ---- conv3d via 3-stage shift-fold ----
        zeros512 = sb.tile([C, 512], F32)
        nc.vector.memset(zeros512, 0.0)
        for ch in range((XPN + 2 + 511) // 512):
            lo = 512 * ch
            ln = min(512, XPN + 2 - lo)
            nc.vector.tensor_copy(out=xp_sb[:, lo:lo + ln], in_=zeros512[:, 0:ln])
        xp4 = xp_sb[:, 0:XPN].rearrange("c (z y x) -> c z y x", z=ZP, y=18)
        xn4 = xn_sb[:, 0:CONVN].rearrange("c (z y x) -> c z y x", z=ZP, y=16)
        for zp in range(ZP):
            dst = xp4[:, zp, 1:17, 1:17]
            if zp == 0:
                nc.vector.tensor_tensor(out=dst, in0=xn4[:, zp],
                                        in1=mlo_sb.rearrange("c (y x) -> c y x", y=16),
                                        op=mybir.AluOpType.mult)
            elif zp == ZP - 1:
                nc.vector.tensor_tensor(out=dst, in0=xn4[:, zp],
                                        in1=mhi_sb.rearrange("c (y x) -> c y x", y=16),
                                        op=mybir.AluOpType.mult)
            else:
                nc.vector.tensor_copy(out=dst, in_=xn4[:, zp])
        # stage1: fold dx (contraction over C), range XPN
        for ch in range(7):
            lo = 512 * ch
            ln = min(512, XPN - lo)
            s1_ps = psA([18, 512], name="s1_ps")
            for dx in range(3):
                mm(s1_ps[:, 0:ln], WcR_sb[dx], xp_sb[:, lo + dx:lo + dx + ln],
                   start=(dx == 0), stop=(dx == 2))
            nc.vector.tensor_copy(out=s1_sb[:, lo:lo + ln], in_=s1_ps[:, 0:ln])
        # stage2a: fold dy
        for ch in range(7):
            lo = 512 * ch
            ln = min(512, XPN - lo)
            s2_ps = psA([6, 512], name="s2_ps")
            for dy in range(3):
                mm(s2_ps[:, 0:ln], E2aR_sb[dy],
                   s1_sb[:, lo + 18 * dy:lo + 18 * dy + ln],
                   start=(dy == 0), stop=(dy == 2))
            nc.vector.tensor_copy(out=s2_sb[:, lo:lo + ln], in_=s2_ps[:, 0:ln])
        # stage2b: fold dz, per output z-slice
        s2v = s2_sb.rearrange("c (z y x) -> c z y x", z=ZP, y=18)
        for zl in range(0, 8, 2):
            cz_ps = psA([2, 512], name="cz_ps")
            for dz in range(3):
                mm(cz_ps.rearrange("c (z y x) -> c z y x", z=2, y=16),
                   E2bR_sb[dz], s2v[:, zl + dz:zl + dz + 2, 0:16, 0:16],
                   start=(dz == 0), stop=(dz == 2))
            nc.vector.tensor_scalar_add(out=cgf_sb[:, 256 * zl:256 * (zl + 2)],
                                        in0=cz_ps, scalar1=bconv_sb)
        dma(out=cgf_d, in_=cgf_sb)

        # ---- gating weights (independent of LN) ----
        nc.vector.tensor_reduce(out=pooled, in_=xT_sb, axis=mybir.AxisListType.X,
                                op=mybir.AluOpType.add)
        nc.vector.tensor_scalar_mul(out=pooledm, in0=pooled, scalar1=1.0 / N)
        wg_ps = psC([1, 3], name="wg_ps")
        mm(wg_ps, pooledm, WgT_sb)
        nc.vector.tensor_tensor(out=wgr, in0=wg_ps, in1=bgr_sb,
                                op=mybir.AluOpType.add)
        nc.scalar.activation(out=we, in_=wgr, func=mybir.ActivationFunctionType.Exp)
        nc.vector.tensor_reduce(out=ws, in_=we, axis=mybir.AxisListType.X,
                                op=mybir.AluOpType.add)
        nc.vector.reciprocal(out=wrec, in_=ws)
        nc.vector.tensor_scalar_mul(out=wn, in0=we, scalar1=wrec)
        wb_ps = psA([C, 3], name="wb_ps")
        mm(wb_ps, ones_row, wn)
        nc.vector.tensor_copy(out=wb_sb, in_=wb_ps)

        # ---- attention (2 query chunks x 32 key tiles) ----
        out_sb = sb.tile([C, NQ], F32, tag="big8k")  # reuses xtok slot
        for c in range(2):
            q_lo = QCH * c
            o_ps = psB([C, QCH], name="o_ps")
            d_ps = psC([1, QCH], name="d_ps")
            p_prev = None
            for j in range(NKT):
                bs = slice(128 * j, 128 * (j + 1))
                s_ps = psA([C, QCH], name="s_ps")
                for h in range(2):
                    hs = slice(512 * h, 512 * (h + 1))
                    qs = slice(q_lo + 512 * h, q_lo + 512 * (h + 1))
                    mm(s_ps[:, hs], kT_bf[:, bs], qT_bf[:, qs])
                p_bf = sc.tile([C, QCH], BF16, tag="pbuf", bufs=3, name="p_bf")
                nc.scalar.activation(out=p_bf, in_=s_ps,
                                     func=mybir.ActivationFunctionType.Exp,
                                     scale=SCALE_C)
                for h in range(2):
                    hs = slice(512 * h, 512 * (h + 1))
                    mm(o_ps[:, hs], vc_bf[:, bs], p_bf[:, hs],
                       start=(j == 0), stop=(j == NKT - 1))
                if j % 2 == 0:
                    p_prev = p_bf
                else:
                    padd = sc.tile([C, QCH], BF16, tag="padd", bufs=2, name="padd")
                    with nc.allow_low_precision("pair-sum of exp tiles; relative "
                                                "error ~1e-4 on softmax denom"):
                        nc.vector.tensor_tensor(out=padd, in0=p_prev, in1=p_bf,
                                                op=mybir.AluOpType.add)
                    for h in range(2):
                        hs = slice(512 * h, 512 * (h + 1))
                        mm(d_ps[0:1, hs], ones_bf, padd[:, hs],
                           start=(j == 1), stop=(j == NKT - 1))
            # fusion: out = x + w0*((P@vc)/d + bvc) + w2*och
            dr_sc = sc.tile([1, QCH], R32, tag="dr", bufs=1, name="dr_sc")
            with nc.allow_low_precision("1/denom rounded to fp32r for the "
                                        "broadcast matmul; ~1e-4 relative"):
                nc.vector.reciprocal(out=dr_sc, in_=d_ps)
            rb_ps = psA([C, QCH], name="rb_ps")
            for h in range(2):
                hs = slice(512 * h, 512 * (h + 1))
                mm(rb_ps[:, hs], ones_row_r, dr_sc[0:1, hs])
            rb_sc = sc.tile([C, QCH], F32, tag="fuseA", bufs=1, name="rb_sc")
            nc.vector.tensor_copy(out=rb_sc, in_=rb_ps)
            t1_sc = sc.tile([C, QCH], F32, tag="fuseB", bufs=1, name="t1_sc")
            nc.vector.tensor_tensor(out=t1_sc, in0=o_ps, in1=rb_sc,
                                    op=mybir.AluOpType.mult)
            t2_sc = sc.tile([C, QCH], F32, tag="fuseA", bufs=1, name="t2_sc")
            nc.vector.tensor_scalar(out=t2_sc, in0=t1_sc, scalar1=bvcc_sb,
                                    scalar2=wb_sb[:, 0:1],
                                    op0=mybir.AluOpType.add,
                                    op1=mybir.AluOpType.mult)
            t3_sc = sc.tile([C, QCH], F32, tag="fuseB", bufs=1, name="t3_sc")
            nc.vector.scalar_tensor_tensor(out=t3_sc, in0=och_sb[:, q_lo:q_lo + QCH],
                                           scalar=wb_sb[:, 2:3], in1=t2_sc,
                                           op0=mybir.AluOpType.mult,
                                           op1=mybir.AluOpType.add)
            nc.vector.tensor_tensor(out=out_sb[:, q_lo:q_lo + QCH], in0=t3_sc,
                                    in1=xT_sb[:, Q0 + q_lo:Q0 + q_lo + QCH],
                                    op=mybir.AluOpType.add)
            dma(out=out_d[:, q_lo:q_lo + QCH], in_=out_sb[:, q_lo:q_lo + QCH])

        ps.release()
        sc.release()
        sb.release()

    nc.finalize()
    return nc


_PROGRAM = None


def _get_program():
    global _PROGRAM
    if _PROGRAM is None:
        _PROGRAM = _build_program()
    return _PROGRAM


def _host_inputs(x, ln_g, ln_b, Wq, bq, Wk, bk, Wvc, bvc, Wvch, bvch,
                 Wconv, bconv, Wg, bg):
    """Build the 8 per-core input dicts (numpy, float32)."""
    f = np.float32
    wpack = np.zeros((C, 1214), f)
    wpack[:, 0:128] = Wq.T
    wpack[:, 128:256] = Wk.T
    wpack[:, 256:384] = Wvc.T
    wpack[:, 384:512] = Wvch.T
    wpack[:, 512:640] = np.eye(C, dtype=f)
    wpack[:, 640:643] = Wg.T
    for dx in range(3):
        wt = np.transpose(Wconv[:, :, :, :, dx], (1, 2, 3, 0)).reshape(C, 18)
        wpack[:, 643 + 18 * dx:661 + 18 * dx] = wt
    # masks filled per-core below (cols 697:1209)
    wpack[:, 1209] = ln_b
    wpack[:, 1210] = bq
    wpack[:, 1211] = bk
    wpack[:, 1212] = bvch
    wpack[:, 1213] = bvc

    spack = np.zeros((18, 540), f)
    for dy in range(3):
        for dz in range(3):
            for o in range(2):
                spack[(dz * 3 + dy) * 2 + o, 6 * dy + dz * 2 + o] = 1.0
    for dz in range(3):
        for o in range(2):
            spack[dz * 2 + o, 18 + 2 * dz + o] = 1.0
    spack[0, 24:152] = ln_g
    spack[1, 24:152] = ln_b
    spack[0, 152:280] = bq
    spack[0, 280:408] = bk
    spack[0, 408:536] = bk * N
    spack[0, 536:539] = bg
    spack[0:2, 539] = bconv

    maps = []
    for core in range(8):
        b, h = core // 2, core % 2
        xb = np.ascontiguousarray(x[b].reshape(C, N), f)
        shift = 256 * (8 * h - 1)
        rot = np.roll(xb, -shift, axis=1)
        wp = wpack.copy()
        wp[:, 697:953] = 1.0 if h == 1 else 0.0    # mask_lo
        wp[:, 953:1209] = 1.0 if h == 0 else 0.0   # mask_hi
        maps.append({"xT": np.ascontiguousarray(rot, f), "wpack": wp,
                     "spack": spack})
    return maps


def kernel(**inputs):
    inputs = {k: np.asarray(v, np.float32) for k, v in inputs.items()}
    nc = _get_program()
    maps = _host_inputs(**inputs)
    res = run_bass_kernel_spmd(nc, maps, core_ids=list(range(8)))
    output = np.empty((B, C, D, H, W), np.float32)
    cgf = np.empty((B, 2, D, H, W), np.float32)
    for core in range(8):
        b, h = core // 2, core % 2
        zs = slice(8 * h, 8 * h + 8)
        output[b, :, zs] = res.results[core]["out"].reshape(C, 8, H, W)
        cgf[b, :, zs] = res.results[core]["cgf"].reshape(2, 8, H, W)
    return output, cgf


# revision 33
# speedup vs baseline: 1.3624x; 1.0025x over previous
"""Trainium2 Bass kernel for the CTG_EPA block (dense transformer).

Shapes: x (4, 128, 16, 16, 16) -> (output (4,128,16,16,16), cgf (4,2,16,16,16)).
Sharding: 8 cores = 4 samples x 2 query-halves (sequence-parallel attention).
Each core receives its sample's full token set (rotated so its query half and
conv halo sit at fixed positions -> SPMD-uniform program) and computes:
  LayerNorm (channel-major, PE column-sum + K=1 broadcast matmuls)
  q/k/vch projections (channel-major), vc (token-major), Gram G = xn^T xn
  full NxN attention for its 2048 queries (S^T layout, softmax transpose-free)
  CxC channel attention via G, gated fusion, residual
  Conv3d C->2 k=3 SAME via 3-stage shift-fold matmuls (dx, dy, dz)
"""

import numpy as np

import concourse.bass as bass
import concourse.tile as tile
from concourse import bacc
from concourse import mybir
from concourse.bass_utils import run_bass_kernel_spmd

F32 = mybir.dt.float32
BF16 = mybir.dt.bfloat16
R32 = mybir.dt.float32r

B, C, D, H, W = 4, 128, 16, 16, 16
N = D * H * W            # 4096 tokens per sample
NQ = 2048                # queries per core
Q0 = 256                 # query offset in rotated token space
QCH = 1024               # attention query chunk
NKT = N // 128           # 32 key tiles
NCH = N // 512           # 8 LN/proj chunks
SCALE_C = 1.0 / float(np.sqrt(C))
SCALE_N = 1.0 / float(np.sqrt(N))
EPS = 1e-5

# padded conv volume (local): 10 z-slices x 18 x 18 (+2 guard)
ZP = 10
PLANE = 18 * 18          # 324
XPN = ZP * PLANE         # 3240
CONVN = 2560             # rotated tokens feeding conv (10 z-slices x 256)


def r32(ap):
    return ap.bitcast(R32)


def _build_program():
    nc = bacc.Bacc(None, target_bir_lowering=False)

    # ---- dram parameters (per-core inputs) ----
    def inp(name, shape):
        return nc.declare_dram_parameter(name, list(shape), F32, isOutput=False)[:]

    xT = inp("xT", (C, N))                 # raw x, channel-major, rotated
    wpack = inp("wpack", (C, 1214))        # packed 128-row weights/constants
    spack = inp("spack", (18, 540))        # packed small-row weights/constants

    out_d = nc.declare_dram_parameter("out", [C, NQ], F32, isOutput=True)[:]
    cgf_d = nc.declare_dram_parameter("cgf", [2, NQ], F32, isOutput=True)[:]

    with tile.TileContext(nc) as tc:
        sb = tc.alloc_tile_pool(name="sb", bufs=1)
        sc = tc.alloc_tile_pool(name="sc", bufs=3)     # small rotating scratch
        # single PSUM pool, three tags -> 4 + 2 + 2 = 8 banks
        ps = tc.alloc_tile_pool(name="ps", bufs=1, space="PSUM")

        def psA(shape, dtype=F32, name="psA_t"):
            return ps.tile(shape, dtype, tag="A", bufs=2, name=name)

        def psB(shape, dtype=F32, name="psB_t"):
            return ps.tile(shape, dtype, tag="B", bufs=1, name=name)

        def psC(shape, dtype=F32, name="psC_t"):
            return ps.tile(shape, dtype, tag="C", bufs=1, name=name)

        # ---- persistent SBUF tensors ----
        xT_sb = sb.tile([C, N], F32)
        xn_sb = sb.tile([C, N], R32)
        kT_bf = sb.tile([C, N], BF16)
        qT_bf = sb.tile([C, NQ], BF16)
        vc_bf = sb.tile([C, N], BF16)         # token-major vc blocks
        xtok_bf = sb.tile([C, N], BF16, tag="big8k")  # token-major xn (Gram)
        xp_sb = sb.tile([C, XPN + 2], R32)    # padded conv input (+guard)
        s1_sb = sb.tile([18, XPN + 40], R32)
        s2_sb = sb.tile([6, XPN], R32)
        cgf_sb = sb.tile([2, NQ], F32)
        xn_bf = sb.tile([C, N], BF16)
        sxrow_sb = sb.tile([1, N], F32)
        sqrow_sb = sb.tile([1, N], F32)
        s32_sb = sb.tile([C, 32], F32)
        sq32_sb = sb.tile([C, 32], F32)
        a32_sb = sb.tile([C, 32], F32)
        d32_sb = sb.tile([C, 32], F32)
        m32_sb = sb.tile([C, 32], F32)
        G_sb = sb.tile([C, C], R32)
        T1_sb = sb.tile([C, C], R32)
        Ae_sb = sb.tile([C, C], F32)
        Asm_sb = sb.tile([C, C], R32)
        AsmT_sb = sb.tile([C, C], R32)
        wb_sb = sb.tile([C, 3], F32)

        # small constants / vectors
        ones_col = sb.tile([C, 1], F32)
        ones_bf = sb.tile([C, 1], BF16)
        ones_row = sb.tile([1, C], F32)
        eps_col = sb.tile([C, 1], F32)
        pooled = sb.tile([C, 1], F32)
        pooledm = sb.tile([C, 1], F32)
        sbar = sb.tile([C, 1], F32)
        rq_sb = sb.tile([1, C], R32)
        rk_sb = sb.tile([1, C], R32)
        asum = sb.tile([C, 1], F32)
        arec = sb.tile([C, 1], F32)
        wgr = sb.tile([1, 3], F32)
        we = sb.tile([1, 3], F32)
        ws = sb.tile([1, 1], F32)
        wrec = sb.tile([1, 1], F32)
        wn = sb.tile([1, 3], F32)

        nc.vector.memset(ones_col, 1.0)
        nc.vector.memset(ones_bf, 1.0)
        nc.vector.memset(ones_row, 1.0)
        nc.vector.memset(eps_col, EPS)

        # ---- input DMAs ----
        dma = nc.sync.dma_start
        for j in range(NCH):
            cs = slice(512 * j, 512 * (j + 1))
            dma(out=xT_sb[:, cs], in_=xT[:, cs])
        wp_sb = sb.tile([C, 1214], F32); dma(out=wp_sb, in_=wpack)
        sp_sb = sb.tile([18, 540], F32); dma(out=sp_sb, in_=spack)
        WqT_sb = wp_sb[:, 0:128]
        WkT_sb = wp_sb[:, 128:256]
        WvcT_sb = wp_sb[:, 256:384]
        WvchT_sb = wp_sb[:, 384:512]
        id_sb = wp_sb[:, 512:640]
        WgT_sb = wp_sb[:, 640:643]
        Wc_sb = [wp_sb[:, 643 + 18 * i:661 + 18 * i] for i in range(3)]
        mlo_sb = wp_sb[:, 697:953]
        mhi_sb = wp_sb[:, 953:1209]
        lnb_col_sb = wp_sb[:, 1209:1210]
        bqc_sb = wp_sb[:, 1210:1211]
        bkc_sb = wp_sb[:, 1211:1212]
        bvchc_sb = wp_sb[:, 1212:1213]
        bvcc_sb = wp_sb[:, 1213:1214]
        E2a_sb = [sp_sb[0:18, 6 * i:6 * i + 6] for i in range(3)]
        E2b_sb = [sp_sb[0:6, 18 + 2 * i:20 + 2 * i] for i in range(3)]
        gb2_sb = sp_sb[0:2, 24:152]
        bqr_sb = sp_sb[0:1, 152:280]
        bkr_sb = sp_sb[0:1, 280:408]
        bkNr_sb = sp_sb[0:1, 408:536]
        bgr_sb = sp_sb[0:1, 536:539]
        bconv_sb = sp_sb[0:2, 539:540]
        idR_sb = sb.tile([C, C], R32)
        nc.vector.tensor_copy(out=idR_sb, in_=id_sb)
        id_bf = sb.tile([C, C], BF16)
        nc.vector.tensor_copy(out=id_bf, in_=id_sb)

        mm = nc.tensor.matmul

        # fp32r-rounded copies of weights consumed by fp32r matmuls
        WqTr_sb = sb.tile([C, C], R32)
        nc.vector.tensor_copy(out=WqTr_sb, in_=WqT_sb)
        WkTr_sb = sb.tile([C, C], R32)
        nc.vector.tensor_copy(out=WkTr_sb, in_=WkT_sb)
        WvcT_bf = sb.tile([C, C], BF16)
        nc.vector.tensor_copy(out=WvcT_bf, in_=WvcT_sb)
        WvchTr_sb = sb.tile([C, C], R32)
        nc.vector.tensor_copy(out=WvchTr_sb, in_=WvchT_sb)
        ones_row_r = sb.tile([1, C], R32)
        nc.vector.tensor_copy(out=ones_row_r, in_=ones_row)
        WcR_sb = []
        for i in range(3):
            t = sb.tile([C, 18], R32, name=f"WcR_sb{i}")
            nc.vector.tensor_copy(out=t, in_=Wc_sb[i])
            WcR_sb.append(t)
        E2aR_sb = []
        for i in range(3):
            t = sb.tile([18, 6], R32, name=f"E2aR_sb{i}")
            nc.vector.tensor_copy(out=t, in_=E2a_sb[i])
            E2aR_sb.append(t)
        E2bR_sb = []
        for i in range(3):
            t = sb.tile([6, 2], R32, name=f"E2bR_sb{i}")
            nc.vector.tensor_copy(out=t, in_=E2b_sb[i])
            E2bR_sb.append(t)

        # ---- LayerNorm over channels ----
        # column sums of x and x^2 via ones-matmul, chunked; hop to (128,32)
        # for parallel stats, hop back to rows for K=1 broadcast matmuls.
        for j in range(NCH):
            cs = slice(512 * j, 512 * (j + 1))
            x2_sc = sc.tile([C, 512], F32, tag="ch512", bufs=2, name="x2_sc")
            nc.vector.tensor_tensor(out=x2_sc, in0=xT_sb[:, cs], in1=xT_sb[:, cs],
                                    op=mybir.AluOpType.mult)
            sx_ps = psA([1, 512], name="sx_ps")
            mm(sx_ps, ones_col, xT_sb[:, cs])
            nc.vector.tensor_copy(out=sxrow_sb[:, cs], in_=sx_ps)
            sq_ps = psA([1, 512], name="sq_ps")
            mm(sq_ps, ones_col, x2_sc)
            nc.vector.tensor_copy(out=sqrow_sb[:, cs], in_=sq_ps)
        dma(out=s32_sb, in_=sxrow_sb[0:1, :].rearrange("a (p f) -> a p f", p=128))
        dma(out=sq32_sb, in_=sqrow_sb[0:1, :].rearrange("a (p f) -> a p f", p=128))
        # stats: var = (Sx2 - Sx^2/C)/C ; s = 1/sqrt(var+eps) ; m = (Sx/C)*s
        nc.vector.scalar_tensor_tensor(out=a32_sb, in0=s32_sb, scalar=1.0 / C,
                                       in1=s32_sb, op0=mybir.AluOpType.mult,
                                       op1=mybir.AluOpType.mult)
        nc.vector.tensor_tensor(out=d32_sb, in0=sq32_sb, in1=a32_sb,
                                op=mybir.AluOpType.subtract)
        nc.scalar.activation(out=d32_sb, in_=d32_sb,
                             func=mybir.ActivationFunctionType.Sqrt,
                             bias=eps_col, scale=1.0 / C)
        nc.vector.reciprocal(out=a32_sb, in_=d32_sb)           # a32 <- rstd s
        nc.vector.scalar_tensor_tensor(out=m32_sb, in0=s32_sb, scalar=1.0 / C,
                                       in1=a32_sb, op0=mybir.AluOpType.mult,
                                       op1=mybir.AluOpType.mult)  # m = mu*s
        dma(out=sxrow_sb[0:1, :].rearrange("a (p f) -> a p f", p=128), in_=a32_sb)
        dma(out=sqrow_sb[0:1, :].rearrange("a (p f) -> a p f", p=128), in_=m32_sb)

        for j in range(NCH):
            cs = slice(512 * j, 512 * (j + 1))
            sb_ps = psA([C, 512], name="sb_ps")
            mm(sb_ps, gb2_sb[0:1, :], sxrow_sb[:, cs])         # g_c * s_n
            mb_ps = psA([C, 512], name="mb_ps")
            mm(mb_ps, gb2_sb[0:1, :], sqrow_sb[:, cs])         # g_c * m_n
            t_sc = sc.tile([C, 512], F32, tag="ch512", bufs=2, name="t_sc")
            nc.vector.tensor_tensor(out=t_sc, in0=xT_sb[:, cs], in1=sb_ps,
                                    op=mybir.AluOpType.mult)
            # xn = (x*g*s + b) - g*m
            nc.vector.scalar_tensor_tensor(out=xn_sb[:, cs], in0=t_sc,
                                           scalar=lnb_col_sb, in1=mb_ps,
                                           op0=mybir.AluOpType.add,
                                           op1=mybir.AluOpType.subtract)
            nc.vector.tensor_copy(out=xn_bf[:, cs], in_=xn_sb[:, cs])

        # ---- projections ----
        for j in range(NCH):
            cs = slice(512 * j, 512 * (j + 1))
            k_ps = psA([C, 512], name="k_ps")
            mm(k_ps, WkTr_sb, xn_sb[:, cs])
            nc.vector.tensor_scalar_add(out=kT_bf[:, cs], in0=k_ps, scalar1=bkc_sb)
        for j in range(NQ // 512):
            cs = slice(512 * j, 512 * (j + 1))
            qs = slice(Q0 + 512 * j, Q0 + 512 * (j + 1))
            q_ps = psA([C, 512], name="q_ps")
            mm(q_ps, WqTr_sb, xn_sb[:, qs])
            nc.vector.tensor_scalar_add(out=qT_bf[:, cs], in0=q_ps, scalar1=bqc_sb)
        for j in range(NKT):
            bs = slice(128 * j, 128 * (j + 1))
            vc_ps = psA([128, C], name="vc_ps")
            mm(vc_ps, xn_bf[:, bs], WvcT_bf)
            nc.vector.tensor_copy(out=vc_bf[:, bs], in_=vc_ps)

        # ---- Gram + channel attention matrix ----
        for j in range(NKT):
            bs = slice(128 * j, 128 * (j + 1))
            tr_ps = psA([128, C], BF16, name="tr_ps")
            nc.tensor.transpose(tr_ps, xn_bf[:, bs], id_bf)
            nc.vector.tensor_copy(out=xtok_bf[:, bs], in_=tr_ps)
        G_ps = psB([C, C], name="G_ps")
        for j in range(NKT):
            bs = slice(128 * j, 128 * (j + 1))
            mm(G_ps, xtok_bf[:, bs], xtok_bf[:, bs],
               start=(j == 0), stop=(j == NKT - 1))
        nc.vector.tensor_copy(out=G_sb, in_=G_ps)
        nc.vector.tensor_reduce(out=sbar, in_=xn_sb.bitcast(F32),
                                axis=mybir.AxisListType.X,
                                op=mybir.AluOpType.add)
        sbar_r = sb.tile([C, 1], R32)
        nc.vector.tensor_copy(out=sbar_r, in_=sbar)
        bqr_r = sb.tile([1, C], R32)
        nc.vector.tensor_copy(out=bqr_r, in_=bqr_sb)
        bkr_r = sb.tile([1, C], R32)
        nc.vector.tensor_copy(out=bkr_r, in_=bkr_sb)
        bkNr_r = sb.tile([1, C], R32)
        nc.vector.tensor_copy(out=bkNr_r, in_=bkNr_sb)
        T1_ps = psA([C, C], name="T1_ps")
        mm(T1_ps, G_sb, WkTr_sb)
        nc.vector.tensor_copy(out=T1_sb, in_=T1_ps)
        rq_ps = psA([1, C], name="rq_ps")
        mm(rq_ps, sbar_r, WqTr_sb)
        nc.vector.tensor_copy(out=rq_sb, in_=rq_ps)
        rk_ps = psA([1, C], name="rk_ps")
        mm(rk_ps, sbar_r, WkTr_sb)
        nc.vector.tensor_copy(out=rk_sb, in_=rk_ps)
        A_ps = psB([C, C], name="A_ps")
        mm(A_ps, WqTr_sb, T1_sb, start=True, stop=False)
        mm(A_ps, rq_sb, bkr_r, start=False, stop=False)
        mm(A_ps, bqr_r, rk_sb, start=False, stop=False)
        mm(A_ps, bqr_r, bkNr_r, start=False, stop=True)
        nc.scalar.activation(out=Ae_sb, in_=A_ps,
                             func=mybir.ActivationFunctionType.Exp,
                             scale=SCALE_N, accum_out=asum)
        nc.vector.reciprocal(out=arec, in_=asum)
        nc.vector.tensor_scalar_mul(out=Asm_sb, in0=Ae_sb, scalar1=arec)
        At_ps = psA([C, C], R32, name="At_ps")
        nc.tensor.transpose(At_ps, Asm_sb, idR_sb)
        nc.vector.tensor_copy(out=AsmT_sb, in_=At_ps)

        # ---- conv3d via 3-stage shift-fold ----
        zeros512 = sb.tile([C, 512], F32)
        nc.vector.memset(zeros512, 0.0)
        for ch in range((XPN + 2 + 511) // 512):
            lo = 512 * ch
            ln = min(512, XPN + 2 - lo)
            nc.vector.tensor_copy(out=xp_sb[:, lo:lo + ln], in_=zeros512[:, 0:ln])
        xp4 = xp_sb[:, 0:XPN].rearrange("c (z y x) -> c z y x", z=ZP, y=18)
        xn4 = xn_sb[:, 0:CONVN].rearrange("c (z y x) -> c z y x", z=ZP, y=16)
        for zp in range(ZP):
            dst = xp4[:, zp, 1:17, 1:17]
            if zp == 0:
                nc.vector.tensor_tensor(out=dst, in0=xn4[:, zp],
                                        in1=mlo_sb.rearrange("c (y x) -> c y x", y=16),
                                        op=mybir.AluOpType.mult)
            elif zp == ZP - 1:
                nc.vector.tensor_tensor(out=dst, in0=xn4[:, zp],
                                        in1=mhi_sb.rearrange("c (y x) -> c y x", y=16),
                                        op=mybir.AluOpType.mult)
            else:
                nc.vector.tensor_copy(out=dst, in_=xn4[:, zp])
        # stage1: fold dx (contraction over C), range XPN
        for ch in range(7):
            lo = 512 * ch
            ln = min(512, XPN - lo)
            s1_ps = psA([18, 512], name="s1_ps")
            for dx in range(3):
                mm(s1_ps[:, 0:ln], WcR_sb[dx], xp_sb[:, lo + dx:lo + dx + ln],
                   start=(dx == 0), stop=(dx == 2))
            nc.vector.tensor_copy(out=s1_sb[:, lo:lo + ln], in_=s1_ps[:, 0:ln])
        # stage2a: fold dy
        for ch in range(7):
            lo = 512 * ch
            ln = min(512, XPN - lo)
            s2_ps = psA([6, 512], name="s2_ps")
            for dy in range(3):
                mm(s2_ps[:, 0:ln], E2aR_sb[dy],
                   s1_sb[:, lo + 18 * dy:lo + 18 * dy + ln],
                   start=(dy == 0), stop=(dy == 2))
            nc.vector.tensor_copy(out=s2_sb[:, lo:lo + ln], in_=s2_ps[:, 0:ln])
        # stage2b: fold dz, per output z-slice
        s2v = s2_sb.rearrange("c (z y x) -> c z y x", z=ZP, y=18)
        for zl in range(0, 8, 2):
            cz_ps = psA([2, 512], name="cz_ps")
            for dz in range(3):
                mm(cz_ps.rearrange("c (z y x) -> c z y x", z=2, y=16),
                   E2bR_sb[dz], s2v[:, zl + dz:zl + dz + 2, 0:16, 0:16],
                   start=(dz == 0), stop=(dz == 2))
            nc.vector.tensor_scalar_add(out=cgf_sb[:, 256 * zl:256 * (zl + 2)],
                                        in0=cz_ps, scalar1=bconv_sb)
        dma(out=cgf_d, in_=cgf_sb)

        # ---- channel-branch output (after conv so its slot reuses s2) ----
        och_sb = sb.tile([C, NQ], F32, tag="s2_sb")
        for j in range(NQ // 512):
            cs = slice(512 * j, 512 * (j + 1))
            qs = slice(Q0 + 512 * j, Q0 + 512 * (j + 1))
            vh_ps = psA([C, 512], name="vh_ps")
            mm(vh_ps, WvchTr_sb, xn_sb[:, qs])
            vch_sc = sc.tile([C, 512], R32, tag="vch512", bufs=2, name="vch_sc")
            nc.vector.tensor_scalar_add(out=vch_sc, in0=vh_ps, scalar1=bvchc_sb)
            oc_ps = psA([C, 512], name="oc_ps")
            mm(oc_ps, AsmT_sb, vch_sc)
            nc.vector.tensor_copy(out=och_sb[:, cs], in_=oc_ps)

        # ---- gating weights (independent of LN) ----
        nc.vector.tensor_reduce(out=pooled, in_=xT_sb, axis=mybir.AxisListType.X,
                                op=mybir.AluOpType.add)
        nc.vector.tensor_scalar_mul(out=pooledm, in0=pooled, scalar1=1.0 / N)
        wg_ps = psC([1, 3], name="wg_ps")
        mm(wg_ps, pooledm, WgT_sb)
        nc.vector.tensor_tensor(out=wgr, in0=wg_ps, in1=bgr_sb,
                                op=mybir.AluOpType.add)
        nc.scalar.activation(out=we, in_=wgr, func=mybir.ActivationFunctionType.Exp)
        nc.vector.tensor_reduce(out=ws, in_=we, axis=mybir.AxisListType.X,
                                op=mybir.AluOpType.add)
        nc.vector.reciprocal(out=wrec, in_=ws)
        nc.vector.tensor_scalar_mul(out=wn, in0=we, scalar1=wrec)
        wb_ps = psA([C, 3], name="wb_ps")
        mm(wb_ps, ones_row, wn)
        nc.vector.tensor_copy(out=wb_sb, in_=wb_ps)

        # ---- attention (2 query chunks x 32 key tiles) ----
        out_sb = sb.tile([C, NQ], F32, tag="big8k")  # reuses xtok slot
        o_sbs, d_sbs = [], []
        for c in range(2):
            q_lo = QCH * c
            o_ps = psB([C, QCH], name="o_ps")
            d_ps = psC([1, QCH], name="d_ps")
            p_prev = None
            for j in range(NKT):
                bs = slice(128 * j, 128 * (j + 1))
                s_ps = psA([C, QCH], name="s_ps")
                for h in range(2):
                    hs = slice(512 * h, 512 * (h + 1))
                    qs = slice(q_lo + 512 * h, q_lo + 512 * (h + 1))
                    mm(s_ps[:, hs], kT_bf[:, bs], qT_bf[:, qs])
                p_bf = sc.tile([C, QCH], BF16, tag="pbuf", bufs=3, name="p_bf")
                nc.scalar.activation(out=p_bf, in_=s_ps,
                                     func=mybir.ActivationFunctionType.Exp,
                                     scale=SCALE_C)
                for h in range(2):
                    hs = slice(512 * h, 512 * (h + 1))
                    mm(o_ps[:, hs], vc_bf[:, bs], p_bf[:, hs],
                       start=(j == 0), stop=(j == NKT - 1))
                if j % 2 == 0:
                    p_prev = p_bf
                else:
                    padd = sc.tile([C, QCH], BF16, tag="padd", bufs=2, name="padd")
                    with nc.allow_low_precision("pair-sum of exp tiles; relative "
                                                "error ~1e-4 on softmax denom"):
                        nc.vector.tensor_tensor(out=padd, in0=p_prev, in1=p_bf,
                                                op=mybir.AluOpType.add)
                    for h in range(2):
                        hs = slice(512 * h, 512 * (h + 1))
                        mm(d_ps[0:1, hs], ones_bf, padd[:, hs],
                           start=(j == 1), stop=(j == NKT - 1))
            # stage accumulators out of PSUM so the next chunk can start
            if c == 0:
                o_sc = sc.tile([C, QCH], F32, tag="ostage0", bufs=1, name="o_sc")
                nc.vector.tensor_copy(out=o_sc, in_=o_ps)
                o_sbs.append(o_sc)
            else:
                o_sbs.append(o_ps)   # last chunk: fusion reads PSUM directly
            dr_sc = sc.tile([1, QCH], R32, tag=f"dstage{c}", bufs=1, name="dr_sc")
            with nc.allow_low_precision("1/denom rounded to fp32r for the "
                                        "broadcast matmul; ~1e-4 relative"):
                nc.vector.reciprocal(out=dr_sc, in_=d_ps)
            d_sbs.append(dr_sc)

        # fusion: out = x + w0*((P@vc)/d + bvc) + w2*och
        for c in range(2):
            q_lo = QCH * c
            rb_ps = psA([C, QCH], name="rb_ps")
            for h in range(2):
                hs = slice(512 * h, 512 * (h + 1))
                mm(rb_ps[:, hs], ones_row_r, d_sbs[c][0:1, hs])
            if c == 0:
                rb_in = rb_ps          # o is in SBUF; one PSUM operand is fine
            else:
                rb_in = sc.tile([C, QCH], F32, tag="fuseA", bufs=1, name="rb_in")
                nc.vector.tensor_copy(out=rb_in, in_=rb_ps)
            t1_sc = sc.tile([C, QCH], F32, tag="fuseB", bufs=1, name="t1_sc")
            nc.vector.tensor_tensor(out=t1_sc, in0=o_sbs[c], in1=rb_in,
                                    op=mybir.AluOpType.mult)
            t2_sc = sc.tile([C, QCH], F32, tag="fuseA", bufs=1, name="t2_sc")
            nc.vector.tensor_scalar(out=t2_sc, in0=t1_sc, scalar1=bvcc_sb,
                                    scalar2=wb_sb[:, 0:1],
                                    op0=mybir.AluOpType.add,
                                    op1=mybir.AluOpType.mult)
            t3_sc = sc.tile([C, QCH], F32, tag="fuseB", bufs=1, name="t3_sc")
            nc.vector.scalar_tensor_tensor(out=t3_sc, in0=och_sb[:, q_lo:q_lo + QCH],
                                           scalar=wb_sb[:, 2:3], in1=t2_sc,
                                           op0=mybir.AluOpType.mult,
                                           op1=mybir.AluOpType.add)
            nc.vector.tensor_tensor(out=out_sb[:, q_lo:q_lo + QCH], in0=t3_sc,
                                    in1=xT_sb[:, Q0 + q_lo:Q0 + q_lo + QCH],
                                    op=mybir.AluOpType.add)
            dma(out=out_d[:, q_lo:q_lo + QCH], in_=out_sb[:, q_lo:q_lo + QCH])

        ps.release()
        sc.release()
        sb.release()

    nc.finalize()
    return nc


_PROGRAM = None


def _get_program():
    global _PROGRAM
    if _PROGRAM is None:
        _PROGRAM = _build_program()
    return _PROGRAM


def _host_inputs(x, ln_g, ln_b, Wq, bq, Wk, bk, Wvc, bvc, Wvch, bvch,
                 Wconv, bconv, Wg, bg):
    """Build the 8 per-core input dicts (numpy, float32)."""
    f = np.float32
    wpack = np.zeros((C, 1214), f)
    wpack[:, 0:128] = Wq.T
    wpack[:, 128:256] = Wk.T
    wpack[:, 256:384] = Wvc.T
    wpack[:, 384:512] = Wvch.T
    wpack[:, 512:640] = np.eye(C, dtype=f)
    wpack[:, 640:643] = Wg.T
    for dx in range(3):
        wt = np.transpose(Wconv[:, :, :, :, dx], (1, 2, 3, 0)).reshape(C, 18)
        wpack[:, 643 + 18 * dx:661 + 18 * dx] = wt
    # masks filled per-core below (cols 697:1209)
    wpack[:, 1209] = ln_b
    wpack[:, 1210] = bq
    wpack[:, 1211] = bk
    wpack[:, 1212] = bvch
    wpack[:, 1213] = bvc

    spack = np.zeros((18, 540), f)
    for dy in range(3):
        for dz in range(3):
            for o in range(2):
                spack[(dz * 3 + dy) * 2 + o, 6 * dy + dz * 2 + o] = 1.0
    for dz in range(3):
        for o in range(2):
            spack[dz * 2 + o, 18 + 2 * dz + o] = 1.0
    spack[0, 24:152] = ln_g
    spack[1, 24:152] = ln_b
    spack[0, 152:280] = bq
    spack[0, 280:408] = bk
    spack[0, 408:536] = bk * N
    spack[0, 536:539] = bg
    spack[0:2, 539] = bconv

    maps = []
    for core in range(8):
        b, h = core // 2, core % 2
        xb = np.ascontiguousarray(x[b].reshape(C, N), f)
        shift = 256 * (8 * h - 1)
        rot = np.roll(xb, -shift, axis=1)
        wp = wpack.copy()
        wp[:, 697:953] = 1.0 if h == 1 else 0.0    # mask_lo
        wp[:, 953:1209] = 1.0 if h == 0 else 0.0   # mask_hi
        maps.append({"xT": np.ascontiguousarray(rot, f), "wpack": wp,
                     "spack": spack})
    return maps


def kernel(**inputs):
    inputs = {k: np.asarray(v, np.float32) for k, v in inputs.items()}
    nc = _get_program()
    maps = _host_inputs(**inputs)
    res = run_bass_kernel_spmd(nc, maps, core_ids=list(range(8)))
    output = np.empty((B, C, D, H, W), np.float32)
    cgf = np.empty((B, 2, D, H, W), np.float32)
    for core in range(8):
        b, h = core // 2, core % 2
        zs = slice(8 * h, 8 * h + 8)
        output[b, :, zs] = res.results[core]["out"].reshape(C, 8, H, W)
        cgf[b, :, zs] = res.results[core]["cgf"].reshape(2, 8, H, W)
    return output, cgf


# revision 35
# speedup vs baseline: 1.3682x; 1.0042x over previous
"""Trainium2 Bass kernel for the CTG_EPA block (dense transformer).

Shapes: x (4, 128, 16, 16, 16) -> (output (4,128,16,16,16), cgf (4,2,16,16,16)).
Sharding: 8 cores = 4 samples x 2 query-halves (sequence-parallel attention).
Each core receives its sample's full token set (rotated so its query half and
conv halo sit at fixed positions -> SPMD-uniform program) and computes:
  LayerNorm (channel-major, PE column-sum + K=1 broadcast matmuls)
  q/k/vch projections (channel-major), vc (token-major), Gram G = xn^T xn
  full NxN attention for its 2048 queries (S^T layout, softmax transpose-free)
  CxC channel attention via G, gated fusion, residual
  Conv3d C->2 k=3 SAME via 3-stage shift-fold matmuls (dx, dy, dz)
"""

import numpy as np

import concourse.bass as bass
import concourse.tile as tile
from concourse.tile_rust import add_dep_helper
from concourse import bacc
from concourse import mybir
from concourse.bass_utils import run_bass_kernel_spmd

F32 = mybir.dt.float32
BF16 = mybir.dt.bfloat16
R32 = mybir.dt.float32r

B, C, D, H, W = 4, 128, 16, 16, 16
N = D * H * W            # 4096 tokens per sample
NQ = 2048                # queries per core
Q0 = 256                 # query offset in rotated token space
QCH = 1024               # attention query chunk
NKT = N // 128           # 32 key tiles
NCH = N // 512           # 8 LN/proj chunks
SCALE_C = 1.0 / float(np.sqrt(C))
SCALE_N = 1.0 / float(np.sqrt(N))
EPS = 1e-5

# padded conv volume (local): 10 z-slices x 18 x 18 (+2 guard)
ZP = 10
PLANE = 18 * 18          # 324
XPN = ZP * PLANE         # 3240
CONVN = 2560             # rotated tokens feeding conv (10 z-slices x 256)


def r32(ap):
    return ap.bitcast(R32)


def _build_program():
    nc = bacc.Bacc(None, target_bir_lowering=False)

    # ---- dram parameters (per-core inputs) ----
    def inp(name, shape):
        return nc.declare_dram_parameter(name, list(shape), F32, isOutput=False)[:]

    xT = inp("xT", (C, N))                 # raw x, channel-major, rotated
    wpack = inp("wpack", (C, 1214))        # packed 128-row weights/constants
    spack = inp("spack", (18, 540))        # packed small-row weights/constants

    out_d = nc.declare_dram_parameter("out", [C, NQ], F32, isOutput=True)[:]
    cgf_d = nc.declare_dram_parameter("cgf", [2, NQ], F32, isOutput=True)[:]

    with tile.TileContext(nc) as tc:
        sb = tc.alloc_tile_pool(name="sb", bufs=1)
        sc = tc.alloc_tile_pool(name="sc", bufs=3)     # small rotating scratch
        # single PSUM pool, three tags -> 4 + 2 + 2 = 8 banks
        ps = tc.alloc_tile_pool(name="ps", bufs=1, space="PSUM")

        def psA(shape, dtype=F32, name="psA_t"):
            return ps.tile(shape, dtype, tag="A", bufs=2, name=name)

        def psB(shape, dtype=F32, name="psB_t"):
            return ps.tile(shape, dtype, tag="B", bufs=1, name=name)

        def psC(shape, dtype=F32, name="psC_t"):
            return ps.tile(shape, dtype, tag="C", bufs=1, name=name)

        # ---- persistent SBUF tensors ----
        xT_sb = sb.tile([C, N], F32)
        xn_sb = sb.tile([C, N], R32)
        kT_bf = sb.tile([C, N], BF16)
        qT_bf = sb.tile([C, NQ], BF16)
        vc_bf = sb.tile([C, N], BF16)         # token-major vc blocks
        xtok_bf = sb.tile([C, N], BF16, tag="big8k")  # token-major xn (Gram)
        xp_sb = sb.tile([C, XPN + 2], R32)    # padded conv input (+guard)
        s1_sb = sb.tile([18, XPN + 40], R32)
        s2_sb = sb.tile([6, XPN], R32)
        cgf_sb = sb.tile([2, NQ], F32)
        xn_bf = sb.tile([C, N], BF16)
        sxrow_sb = sb.tile([1, N], F32)
        sqrow_sb = sb.tile([1, N], F32)
        s32_sb = sb.tile([C, 32], F32)
        sq32_sb = sb.tile([C, 32], F32)
        a32_sb = sb.tile([C, 32], F32)
        d32_sb = sb.tile([C, 32], F32)
        m32_sb = sb.tile([C, 32], F32)
        G_sb = sb.tile([C, C], R32)
        T1_sb = sb.tile([C, C], R32)
        Ae_sb = sb.tile([C, C], F32)
        Asm_sb = sb.tile([C, C], R32)
        AsmT_sb = sb.tile([C, C], R32)
        wb_sb = sb.tile([C, 3], F32)

        # small constants / vectors
        ones_col = sb.tile([C, 1], F32)
        ones_bf = sb.tile([C, 1], BF16)
        ones_row = sb.tile([1, C], F32)
        eps_col = sb.tile([C, 1], F32)
        pooled = sb.tile([C, 1], F32)
        pooledm = sb.tile([C, 1], F32)
        sbar = sb.tile([C, 1], F32)
        rq_sb = sb.tile([1, C], R32)
        rk_sb = sb.tile([1, C], R32)
        asum = sb.tile([C, 1], F32)
        arec = sb.tile([C, 1], F32)
        wgr = sb.tile([1, 3], F32)
        we = sb.tile([1, 3], F32)
        ws = sb.tile([1, 1], F32)
        wrec = sb.tile([1, 1], F32)
        wn = sb.tile([1, 3], F32)

        nc.vector.memset(ones_col, 1.0)
        nc.vector.memset(ones_bf, 1.0)
        nc.vector.memset(ones_row, 1.0)
        nc.vector.memset(eps_col, EPS)

        # ---- input DMAs ----
        dma = nc.sync.dma_start
        for j in range(NCH):
            cs = slice(512 * j, 512 * (j + 1))
            dma(out=xT_sb[:, cs], in_=xT[:, cs])
        wp_sb = sb.tile([C, 1214], F32); dma(out=wp_sb, in_=wpack)
        sp_sb = sb.tile([18, 540], F32); dma(out=sp_sb, in_=spack)
        WqT_sb = wp_sb[:, 0:128]
        WkT_sb = wp_sb[:, 128:256]
        WvcT_sb = wp_sb[:, 256:384]
        WvchT_sb = wp_sb[:, 384:512]
        id_sb = wp_sb[:, 512:640]
        WgT_sb = wp_sb[:, 640:643]
        Wc_sb = [wp_sb[:, 643 + 18 * i:661 + 18 * i] for i in range(3)]
        mlo_sb = wp_sb[:, 697:953]
        mhi_sb = wp_sb[:, 953:1209]
        lnb_col_sb = wp_sb[:, 1209:1210]
        bqc_sb = wp_sb[:, 1210:1211]
        bkc_sb = wp_sb[:, 1211:1212]
        bvchc_sb = wp_sb[:, 1212:1213]
        bvcc_sb = wp_sb[:, 1213:1214]
        E2a_sb = [sp_sb[0:18, 6 * i:6 * i + 6] for i in range(3)]
        E2b_sb = [sp_sb[0:6, 18 + 2 * i:20 + 2 * i] for i in range(3)]
        gb2_sb = sp_sb[0:2, 24:152]
        bqr_sb = sp_sb[0:1, 152:280]
        bkr_sb = sp_sb[0:1, 280:408]
        bkNr_sb = sp_sb[0:1, 408:536]
        bgr_sb = sp_sb[0:1, 536:539]
        bconv_sb = sp_sb[0:2, 539:540]
        idR_sb = sb.tile([C, C], R32)
        nc.vector.tensor_copy(out=idR_sb, in_=id_sb)
        id_bf = sb.tile([C, C], BF16)
        nc.vector.tensor_copy(out=id_bf, in_=id_sb)

        mm = nc.tensor.matmul

        # fp32r-rounded copies of weights consumed by fp32r matmuls
        WqTr_sb = sb.tile([C, C], R32)
        nc.vector.tensor_copy(out=WqTr_sb, in_=WqT_sb)
        WkTr_sb = sb.tile([C, C], R32)
        nc.vector.tensor_copy(out=WkTr_sb, in_=WkT_sb)
        WvcT_bf = sb.tile([C, C], BF16)
        nc.vector.tensor_copy(out=WvcT_bf, in_=WvcT_sb)
        WvchTr_sb = sb.tile([C, C], R32)
        nc.vector.tensor_copy(out=WvchTr_sb, in_=WvchT_sb)
        ones_row_r = sb.tile([1, C], R32)
        nc.vector.tensor_copy(out=ones_row_r, in_=ones_row)
        WcR_sb = []
        for i in range(3):
            t = sb.tile([C, 18], R32, name=f"WcR_sb{i}")
            nc.vector.tensor_copy(out=t, in_=Wc_sb[i])
            WcR_sb.append(t)
        E2aR_sb = []
        for i in range(3):
            t = sb.tile([18, 6], R32, name=f"E2aR_sb{i}")
            nc.vector.tensor_copy(out=t, in_=E2a_sb[i])
            E2aR_sb.append(t)
        E2bR_sb = []
        for i in range(3):
            t = sb.tile([6, 2], R32, name=f"E2bR_sb{i}")
            nc.vector.tensor_copy(out=t, in_=E2b_sb[i])
            E2bR_sb.append(t)

        # ---- LayerNorm over channels ----
        # column sums of x and x^2 via ones-matmul, chunked; hop to (128,32)
        # for parallel stats, hop back to rows for K=1 broadcast matmuls.
        for j in range(NCH):
            cs = slice(512 * j, 512 * (j + 1))
            x2_sc = sc.tile([C, 512], F32, tag="ch512", bufs=2, name="x2_sc")
            nc.vector.tensor_tensor(out=x2_sc, in0=xT_sb[:, cs], in1=xT_sb[:, cs],
                                    op=mybir.AluOpType.mult)
            sx_ps = psA([1, 512], name="sx_ps")
            mm(sx_ps, ones_col, xT_sb[:, cs])
            nc.vector.tensor_copy(out=sxrow_sb[:, cs], in_=sx_ps)
            sq_ps = psA([1, 512], name="sq_ps")
            mm(sq_ps, ones_col, x2_sc)
            nc.vector.tensor_copy(out=sqrow_sb[:, cs], in_=sq_ps)
        dma(out=s32_sb, in_=sxrow_sb[0:1, :].rearrange("a (p f) -> a p f", p=128))
        dma(out=sq32_sb, in_=sqrow_sb[0:1, :].rearrange("a (p f) -> a p f", p=128))
        # stats: var = (Sx2 - Sx^2/C)/C ; s = 1/sqrt(var+eps) ; m = (Sx/C)*s
        nc.vector.scalar_tensor_tensor(out=a32_sb, in0=s32_sb, scalar=1.0 / C,
                                       in1=s32_sb, op0=mybir.AluOpType.mult,
                                       op1=mybir.AluOpType.mult)
        nc.vector.tensor_tensor(out=d32_sb, in0=sq32_sb, in1=a32_sb,
                                op=mybir.AluOpType.subtract)
        nc.scalar.activation(out=d32_sb, in_=d32_sb,
                             func=mybir.ActivationFunctionType.Sqrt,
                             bias=eps_col, scale=1.0 / C)
        nc.vector.reciprocal(out=a32_sb, in_=d32_sb)           # a32 <- rstd s
        nc.vector.scalar_tensor_tensor(out=m32_sb, in0=s32_sb, scalar=1.0 / C,
                                       in1=a32_sb, op0=mybir.AluOpType.mult,
                                       op1=mybir.AluOpType.mult)  # m = mu*s
        dma(out=sxrow_sb[0:1, :].rearrange("a (p f) -> a p f", p=128), in_=a32_sb)
        dma(out=sqrow_sb[0:1, :].rearrange("a (p f) -> a p f", p=128), in_=m32_sb)

        for j in range(NCH):
            cs = slice(512 * j, 512 * (j + 1))
            sb_ps = psA([C, 512], name="sb_ps")
            mm(sb_ps, gb2_sb[0:1, :], sxrow_sb[:, cs])         # g_c * s_n
            mb_ps = psA([C, 512], name="mb_ps")
            mm(mb_ps, gb2_sb[0:1, :], sqrow_sb[:, cs])         # g_c * m_n
            t_sc = sc.tile([C, 512], F32, tag="ch512", bufs=2, name="t_sc")
            nc.vector.tensor_tensor(out=t_sc, in0=xT_sb[:, cs], in1=sb_ps,
                                    op=mybir.AluOpType.mult)
            # xn = (x*g*s + b) - g*m
            nc.vector.scalar_tensor_tensor(out=xn_sb[:, cs], in0=t_sc,
                                           scalar=lnb_col_sb, in1=mb_ps,
                                           op0=mybir.AluOpType.add,
                                           op1=mybir.AluOpType.subtract)
            nc.scalar.copy(out=xn_bf[:, cs], in_=xn_sb[:, cs])

        # ---- projections ----
        for j in range(NCH):
            cs = slice(512 * j, 512 * (j + 1))
            k_ps = psA([C, 512], name="k_ps")
            mm(k_ps, WkTr_sb, xn_sb[:, cs])
            nc.scalar.activation(out=kT_bf[:, cs], in_=k_ps,
                                 func=mybir.ActivationFunctionType.Identity,
                                 bias=bkc_sb)
        for j in range(NQ // 512):
            cs = slice(512 * j, 512 * (j + 1))
            qs = slice(Q0 + 512 * j, Q0 + 512 * (j + 1))
            q_ps = psA([C, 512], name="q_ps")
            mm(q_ps, WqTr_sb, xn_sb[:, qs])
            nc.scalar.activation(out=qT_bf[:, cs], in_=q_ps,
                                 func=mybir.ActivationFunctionType.Identity,
                                 bias=bqc_sb)
        for j in range(NKT):
            bs = slice(128 * j, 128 * (j + 1))
            vc_ps = psA([128, C], name="vc_ps")
            mm(vc_ps, xn_bf[:, bs], WvcT_bf)
            nc.scalar.copy(out=vc_bf[:, bs], in_=vc_ps)

        # ---- Gram + channel attention matrix ----
        for j in range(NKT):
            bs = slice(128 * j, 128 * (j + 1))
            tr_ps = psA([128, C], BF16, name="tr_ps")
            nc.tensor.transpose(tr_ps, xn_bf[:, bs], id_bf)
            nc.scalar.copy(out=xtok_bf[:, bs], in_=tr_ps)
        G_ps = psB([C, C], name="G_ps")
        for j in range(NKT):
            bs = slice(128 * j, 128 * (j + 1))
            mm(G_ps, xtok_bf[:, bs], xtok_bf[:, bs],
               start=(j == 0), stop=(j == NKT - 1))
        nc.vector.tensor_copy(out=G_sb, in_=G_ps)
        nc.vector.tensor_reduce(out=sbar, in_=xn_sb.bitcast(F32),
                                axis=mybir.AxisListType.X,
                                op=mybir.AluOpType.add)
        sbar_r = sb.tile([C, 1], R32)
        nc.vector.tensor_copy(out=sbar_r, in_=sbar)
        bqr_r = sb.tile([1, C], R32)
        nc.vector.tensor_copy(out=bqr_r, in_=bqr_sb)
        bkr_r = sb.tile([1, C], R32)
        nc.vector.tensor_copy(out=bkr_r, in_=bkr_sb)
        bkNr_r = sb.tile([1, C], R32)
        nc.vector.tensor_copy(out=bkNr_r, in_=bkNr_sb)
        T1_ps = psA([C, C], name="T1_ps")
        mm(T1_ps, G_sb, WkTr_sb)
        nc.vector.tensor_copy(out=T1_sb, in_=T1_ps)
        rq_ps = psA([1, C], name="rq_ps")
        mm(rq_ps, sbar_r, WqTr_sb)
        nc.vector.tensor_copy(out=rq_sb, in_=rq_ps)
        rk_ps = psA([1, C], name="rk_ps")
        mm(rk_ps, sbar_r, WkTr_sb)
        nc.vector.tensor_copy(out=rk_sb, in_=rk_ps)
        A_ps = psB([C, C], name="A_ps")
        mm(A_ps, WqTr_sb, T1_sb, start=True, stop=False)
        mm(A_ps, rq_sb, bkr_r, start=False, stop=False)
        mm(A_ps, bqr_r, rk_sb, start=False, stop=False)
        mm(A_ps, bqr_r, bkNr_r, start=False, stop=True)
        nc.scalar.activation(out=Ae_sb, in_=A_ps,
                             func=mybir.ActivationFunctionType.Exp,
                             scale=SCALE_N, accum_out=asum)
        nc.vector.reciprocal(out=arec, in_=asum)
        nc.vector.tensor_scalar_mul(out=Asm_sb, in0=Ae_sb, scalar1=arec)
        At_ps = psA([C, C], R32, name="At_ps")
        nc.tensor.transpose(At_ps, Asm_sb, idR_sb)
        nc.vector.tensor_copy(out=AsmT_sb, in_=At_ps)

        # ---- conv3d via 3-stage shift-fold ----
        zeros512 = sb.tile([C, 512], F32)
        nc.vector.memset(zeros512, 0.0)
        for ch in range((XPN + 2 + 511) // 512):
            lo = 512 * ch
            ln = min(512, XPN + 2 - lo)
            nc.vector.tensor_copy(out=xp_sb[:, lo:lo + ln], in_=zeros512[:, 0:ln])
        xp4 = xp_sb[:, 0:XPN].rearrange("c (z y x) -> c z y x", z=ZP, y=18)
        xn4 = xn_sb[:, 0:CONVN].rearrange("c (z y x) -> c z y x", z=ZP, y=16)
        for zp in range(ZP):
            dst = xp4[:, zp, 1:17, 1:17]
            if zp == 0:
                nc.vector.tensor_tensor(out=dst, in0=xn4[:, zp],
                                        in1=mlo_sb.rearrange("c (y x) -> c y x", y=16),
                                        op=mybir.AluOpType.mult)
            elif zp == ZP - 1:
                nc.vector.tensor_tensor(out=dst, in0=xn4[:, zp],
                                        in1=mhi_sb.rearrange("c (y x) -> c y x", y=16),
                                        op=mybir.AluOpType.mult)
            else:
                nc.vector.tensor_copy(out=dst, in_=xn4[:, zp])
        # stage1: fold dx (contraction over C), range XPN
        for ch in range(7):
            lo = 512 * ch
            ln = min(512, XPN - lo)
            s1_ps = psA([18, 512], name="s1_ps")
            for dx in range(3):
                mm(s1_ps[:, 0:ln], WcR_sb[dx], xp_sb[:, lo + dx:lo + dx + ln],
                   start=(dx == 0), stop=(dx == 2))
            nc.vector.tensor_copy(out=s1_sb[:, lo:lo + ln], in_=s1_ps[:, 0:ln])
        # stage2a: fold dy
        for ch in range(7):
            lo = 512 * ch
            ln = min(512, XPN - lo)
            s2_ps = psA([6, 512], name="s2_ps")
            for dy in range(3):
                mm(s2_ps[:, 0:ln], E2aR_sb[dy],
                   s1_sb[:, lo + 18 * dy:lo + 18 * dy + ln],
                   start=(dy == 0), stop=(dy == 2))
            nc.vector.tensor_copy(out=s2_sb[:, lo:lo + ln], in_=s2_ps[:, 0:ln])
        # stage2b: fold dz, per output z-slice
        s2v = s2_sb.rearrange("c (z y x) -> c z y x", z=ZP, y=18)
        for zl in range(0, 8, 2):
            cz_ps = psA([2, 512], name="cz_ps")
            for dz in range(3):
                mm(cz_ps.rearrange("c (z y x) -> c z y x", z=2, y=16),
                   E2bR_sb[dz], s2v[:, zl + dz:zl + dz + 2, 0:16, 0:16],
                   start=(dz == 0), stop=(dz == 2))
            nc.vector.tensor_scalar_add(out=cgf_sb[:, 256 * zl:256 * (zl + 2)],
                                        in0=cz_ps, scalar1=bconv_sb)
        conv_done = dma(out=cgf_d, in_=cgf_sb)

        # ---- gating weights (independent of LN) ----
        pooled_inst = nc.vector.tensor_reduce(out=pooled, in_=xT_sb,
                                              axis=mybir.AxisListType.X,
                                              op=mybir.AluOpType.add)
        add_dep_helper(pooled_inst.ins, conv_done.ins, sync=False,
                       reason="keep gating off the LN-phase critical path")
        nc.vector.tensor_scalar_mul(out=pooledm, in0=pooled, scalar1=1.0 / N)
        wg_ps = psC([1, 3], name="wg_ps")
        mm(wg_ps, pooledm, WgT_sb)
        nc.vector.tensor_tensor(out=wgr, in0=wg_ps, in1=bgr_sb,
                                op=mybir.AluOpType.add)
        nc.scalar.activation(out=we, in_=wgr, func=mybir.ActivationFunctionType.Exp)
        nc.vector.tensor_reduce(out=ws, in_=we, axis=mybir.AxisListType.X,
                                op=mybir.AluOpType.add)
        nc.vector.reciprocal(out=wrec, in_=ws)
        nc.vector.tensor_scalar_mul(out=wn, in0=we, scalar1=wrec)
        wb_ps = psA([C, 3], name="wb_ps")
        mm(wb_ps, ones_row, wn)
        nc.vector.tensor_copy(out=wb_sb, in_=wb_ps)

        # ---- channel-branch output (after conv so its slot reuses s2) ----
        och_sb = sb.tile([C, NQ], F32, tag="s2_sb")
        for j in range(NQ // 512):
            cs = slice(512 * j, 512 * (j + 1))
            qs = slice(Q0 + 512 * j, Q0 + 512 * (j + 1))
            vh_ps = psA([C, 512], name="vh_ps")
            mm(vh_ps, WvchTr_sb, xn_sb[:, qs])
            vch_sc = sc.tile([C, 512], R32, tag="vch512", bufs=2, name="vch_sc")
            nc.vector.tensor_scalar_add(out=vch_sc, in0=vh_ps, scalar1=bvchc_sb)
            oc_ps = psA([C, 512], name="oc_ps")
            mm(oc_ps, AsmT_sb, vch_sc)
            nc.vector.scalar_tensor_tensor(out=och_sb[:, cs], in0=oc_ps,
                                           scalar=wb_sb[:, 2:3],
                                           in1=xT_sb[:, qs],
                                           op0=mybir.AluOpType.mult,
                                           op1=mybir.AluOpType.add)

        # ---- attention (2 query chunks x 32 key tiles) ----
        out_sb = sb.tile([C, NQ], F32, tag="big8k")  # reuses xtok slot
        o_sbs, d_sbs = [], []
        for c in range(2):
            q_lo = QCH * c
            o_ps = psB([C, QCH], name="o_ps")
            d_ps = psC([1, QCH], name="d_ps")
            p_prev = None
            for j in range(NKT):
                bs = slice(128 * j, 128 * (j + 1))
                s_ps = psA([C, QCH], name="s_ps")
                for h in range(2):
                    hs = slice(512 * h, 512 * (h + 1))
                    qs = slice(q_lo + 512 * h, q_lo + 512 * (h + 1))
                    mm(s_ps[:, hs], kT_bf[:, bs], qT_bf[:, qs])
                p_bf = sc.tile([C, QCH], BF16, tag="pbuf", bufs=3, name="p_bf")
                nc.scalar.activation(out=p_bf, in_=s_ps,
                                     func=mybir.ActivationFunctionType.Exp,
                                     scale=SCALE_C)
                for h in range(2):
                    hs = slice(512 * h, 512 * (h + 1))
                    mm(o_ps[:, hs], vc_bf[:, bs], p_bf[:, hs],
                       start=(j == 0), stop=(j == NKT - 1))
                if j % 2 == 0:
                    p_prev = p_bf
                else:
                    padd = sc.tile([C, QCH], BF16, tag="padd", bufs=2, name="padd")
                    with nc.allow_low_precision("pair-sum of exp tiles; relative "
                                                "error ~1e-4 on softmax denom"):
                        nc.vector.tensor_tensor(out=padd, in0=p_prev, in1=p_bf,
                                                op=mybir.AluOpType.add)
                    for h in range(2):
                        hs = slice(512 * h, 512 * (h + 1))
                        mm(d_ps[0:1, hs], ones_bf, padd[:, hs],
                           start=(j == 1), stop=(j == NKT - 1))
            # stage accumulators out of PSUM so the next chunk can start
            if c == 0:
                o_sc = sc.tile([C, QCH], F32, tag="ostage0", bufs=1, name="o_sc")
                nc.vector.tensor_copy(out=o_sc, in_=o_ps)
                o_sbs.append(o_sc)
            else:
                o_sbs.append(o_ps)   # last chunk: fusion reads PSUM directly
            dr_sc = sc.tile([1, QCH], R32, tag=f"dstage{c}", bufs=1, name="dr_sc")
            with nc.allow_low_precision("1/denom rounded to fp32r for the "
                                        "broadcast matmul; ~1e-4 relative"):
                nc.vector.reciprocal(out=dr_sc, in_=d_ps)
            d_sbs.append(dr_sc)

        # fusion: out = x + w0*((P@vc)/d + bvc) + w2*och
        for c in range(2):
            q_lo = QCH * c
            rb_ps = psA([C, QCH], name="rb_ps")
            for h in range(2):
                hs = slice(512 * h, 512 * (h + 1))
                mm(rb_ps[:, hs], ones_row_r, d_sbs[c][0:1, hs])
            if c == 0:
                rb_in = rb_ps          # o is in SBUF; one PSUM operand is fine
            else:
                rb_in = sc.tile([C, QCH], F32, tag="fuseA", bufs=1, name="rb_in")
                nc.vector.tensor_copy(out=rb_in, in_=rb_ps)
            t1_sc = sc.tile([C, QCH], F32, tag="fuseB", bufs=1, name="t1_sc")
            nc.vector.tensor_tensor(out=t1_sc, in0=o_sbs[c], in1=rb_in,
                                    op=mybir.AluOpType.mult)
            t2_sc = sc.tile([C, QCH], F32, tag="fuseA", bufs=1, name="t2_sc")
            nc.vector.tensor_scalar(out=t2_sc, in0=t1_sc, scalar1=bvcc_sb,
                                    scalar2=wb_sb[:, 0:1],
                                    op0=mybir.AluOpType.add,
                                    op1=mybir.AluOpType.mult)
            nc.vector.tensor_tensor(out=out_sb[:, q_lo:q_lo + QCH], in0=t2_sc,
                                    in1=och_sb[:, q_lo:q_lo + QCH],
                                    op=mybir.AluOpType.add)
            dma(out=out_d[:, q_lo:q_lo + QCH], in_=out_sb[:, q_lo:q_lo + QCH])

        ps.release()
        sc.release()
        sb.release()

    nc.finalize()
    return nc


_PROGRAM = None


def _get_program():
    global _PROGRAM
    if _PROGRAM is None:
        _PROGRAM = _build_program()
    return _PROGRAM


def _host_inputs(x, ln_g, ln_b, Wq, bq, Wk, bk, Wvc, bvc, Wvch, bvch,
                 Wconv, bconv, Wg, bg):
    """Build the 8 per-core input dicts (numpy, float32)."""
    f = np.float32
    wpack = np.zeros((C, 1214), f)
    wpack[:, 0:128] = Wq.T
    wpack[:, 128:256] = Wk.T
    wpack[:, 256:384] = Wvc.T
    wpack[:, 384:512] = Wvch.T
    wpack[:, 512:640] = np.eye(C, dtype=f)
    wpack[:, 640:643] = Wg.T
    for dx in range(3):
        wt = np.transpose(Wconv[:, :, :, :, dx], (1, 2, 3, 0)).reshape(C, 18)
        wpack[:, 643 + 18 * dx:661 + 18 * dx] = wt
    # masks filled per-core below (cols 697:1209)
    wpack[:, 1209] = ln_b
    wpack[:, 1210] = bq
    wpack[:, 1211] = bk
    wpack[:, 1212] = bvch
    wpack[:, 1213] = bvc

    spack = np.zeros((18, 540), f)
    for dy in range(3):
        for dz in range(3):
            for o in range(2):
                spack[(dz * 3 + dy) * 2 + o, 6 * dy + dz * 2 + o] = 1.0
    for dz in range(3):
        for o in range(2):
            spack[dz * 2 + o, 18 + 2 * dz + o] = 1.0
    spack[0, 24:152] = ln_g
    spack[1, 24:152] = ln_b
    spack[0, 152:280] = bq
    spack[0, 280:408] = bk
    spack[0, 408:536] = bk * N
    spack[0, 536:539] = bg
    spack[0:2, 539] = bconv

    maps = []
    for core in range(8):
        b, h = core // 2, core % 2
        xb = np.ascontiguousarray(x[b].reshape(C, N), f)
        shift = 256 * (8 * h - 1)
        rot = np.roll(xb, -shift, axis=1)
        wp = wpack.copy()
        wp[:, 697:953] = 1.0 if h == 1 else 0.0    # mask_lo
        wp[:, 953:1209] = 1.0 if h == 0 else 0.0   # mask_hi
        maps.append({"xT": np.ascontiguousarray(rot, f), "wpack": wp,
                     "spack": spack})
    return maps


def kernel(**inputs):
    inputs = {k: np.asarray(v, np.float32) for k, v in inputs.items()}
    nc = _get_program()
    maps = _host_inputs(**inputs)
    res = run_bass_kernel_spmd(nc, maps, core_ids=list(range(8)))
    output = np.empty((B, C, D, H, W), np.float32)
    cgf = np.empty((B, 2, D, H, W), np.float32)
    for core in range(8):
        b, h = core // 2, core % 2
        zs = slice(8 * h, 8 * h + 8)
        output[b, :, zs] = res.results[core]["out"].reshape(C, 8, H, W)
        cgf[b, :, zs] = res.results[core]["cgf"].reshape(2, 8, H, W)
    return output, cgf
